# revision 13
# baseline (speedup 1.0000x reference)
"""Trainium2 Bass kernel for a 6-layer post-BatchNorm transformer encoder.

Reference model:
  x = emb[seq] + pes                                  # [B,S,D] = [4,512,1024]
  6x: x = BN(x + attn(x)); x = BN(x + ffn(x))
  BN = per-channel batch stats over (B,S), eps=1e-3.

Sharding: dp=4 x tp=2 mesh over 8 NeuronCores. Core c owns sample b=c//2
(512 tokens) and tensor-parallel half t=c%2 (8 heads of QKV/Wo, 2048 of the
4096 FFN hidden units). Per sublayer the pair AllReduces its partial [D,512]
output in two half-token chunks (bf16, 0.5MB each) so the first chunk's
reduce overlaps the second chunk's matmuls, and the residual-add plus
BN-stat computation of chunk 0 overlaps the reduce of chunk 1. BatchNorm
batch statistics are combined with an 8KB 8-core AllReduce (each sample
counted twice -> divide by 2T).

x is kept in fp32 (residual adds, BN stats and apply are exact); a bf16 copy
feeds the PE. Weights are bf16 (host-converted). V is produced directly in
token-major layout by using x-tiles as the stationary operand (no PE
transposes); its bias comes from a ones-row rank-1 matmul. Attention per
head: scores^T = K_h @ Q_h^T, E = exp(scale*scores^T), U^T = V_h^T @ E^T
with softmax denominators accumulated via a ones-column appended to V,
inverted with the 1-instruction approx reciprocal and broadcast across
partitions by a rank-1 PE matmul.

Host side shards inputs per core and reassembles the 4 samples from cores
0,2,4,6 - no final gather collective.
"""

import os

import numpy as np

import concourse.bass as bass
import concourse.mybir as mybir
import concourse.tile as tile
from concourse import bacc
from concourse.bass import ts
from concourse.masks import make_identity

# ---------------------------------------------------------------- dims
V, D, L, H, B, S = 32000, 1024, 6, 16, 4, 512
HD = D // H            # 64
DF = 4 * D             # 4096
EPS = 1e-3
NC = 8                 # cores
P = 128                # partitions
T = B * S              # 2048 tokens total
SL = S                 # tokens per core (one sample)
CH2 = SL // 2          # AR chunk = 256 tokens
DT = D // P            # 8 d-tiles
TP = 2                 # tensor-parallel width
DSH = D // TP          # qkv out shard = 512
QT = DSH // P          # 4 q-tiles
HPC = H // TP          # heads per core = 8
FSH = DF // TP         # ffn hidden shard = 2048
FMT = FSH // P         # ffn1 m-tiles = 16
KVT = SL // P          # kv token tiles = 4

f32 = mybir.dt.float32
bf16 = mybir.dt.bfloat16
i16 = mybir.dt.int16
AF = mybir.ActivationFunctionType
ALU = mybir.AluOpType

PAIRS = [[0, 1], [2, 3], [4, 5], [6, 7]]
ALL8 = [list(range(NC))]

N_LAYERS = int(os.environ.get("TRN_KERNEL_LAYERS", str(L)))


def build_module(n_layers=None):
    if n_layers is None:
        n_layers = N_LAYERS
    nc = bacc.Bacc("TRN2", target_bir_lowering=False, debug=False,
                   num_devices=NC)

    dt_ = nc.dram_tensor
    io = {
        "emb": dt_("emb", [V, D], f32, kind="ExternalInput").ap(),
        "idx": dt_("idx", [16, SL // 16], i16, kind="ExternalInput").ap(),
        "pesT": dt_("pesT", [D, SL], f32, kind="ExternalInput").ap(),
        "wq": dt_("wq", [L, D, DSH], bf16, kind="ExternalInput").ap(),
        "wk": dt_("wk", [L, D, DSH], bf16, kind="ExternalInput").ap(),
        "wv": dt_("wv", [L, D, DSH], bf16, kind="ExternalInput").ap(),
        "wo": dt_("wo", [L, DSH, D], bf16, kind="ExternalInput").ap(),
        "w1": dt_("w1", [L, D, FSH], bf16, kind="ExternalInput").ap(),
        "w2": dt_("w2", [L, FSH, D], bf16, kind="ExternalInput").ap(),
        "bq": dt_("bq", [L, DSH], f32, kind="ExternalInput").ap(),
        "bk": dt_("bk", [L, DSH], f32, kind="ExternalInput").ap(),
        "bv": dt_("bv", [L, DSH], f32, kind="ExternalInput").ap(),
        "b1": dt_("b1", [L, FSH], f32, kind="ExternalInput").ap(),
        "g1": dt_("g1", [L, D], f32, kind="ExternalInput").ap(),
        "be1": dt_("be1", [L, D], f32, kind="ExternalInput").ap(),
        "g2": dt_("g2", [L, D], f32, kind="ExternalInput").ap(),
        "be2": dt_("be2", [L, D], f32, kind="ExternalInput").ap(),
        "out": dt_("out", [D, SL], f32, kind="ExternalOutput").ap(),
    }

    with tile.TileContext(nc) as tc:
        _build(tc, n_layers, io)
    nc.compile()
    return nc


def _build(tc, n_layers, io):
    from contextlib import ExitStack
    nc = tc.nc
    att_scale = 1.0 / np.sqrt(HD)

    # ------------------------------------------------ pools
    st = ExitStack()
    persist = st.enter_context(tc.tile_pool(name="persist", bufs=1))
    wqkv = st.enter_context(tc.tile_pool(name="wqkv", bufs=1))
    wff = st.enter_context(tc.tile_pool(name="wff", bufs=1))
    small = st.enter_context(tc.tile_pool(name="small", bufs=2))
    ybuf = st.enter_context(tc.tile_pool(name="ybuf", bufs=2))   # AR readback
    gbuf = st.enter_context(tc.tile_pool(name="gbuf", bufs=1))   # emb gather
    epool = st.enter_context(tc.tile_pool(name="epool", bufs=6))
    efpool = st.enter_context(tc.tile_pool(name="efpool", bufs=3))
    hpool = st.enter_context(tc.tile_pool(name="hpool", bufs=1))  # ffn hidden
    psA = st.enter_context(tc.tile_pool(name="psA", bufs=4, space="PSUM"))
    psB = st.enter_context(tc.tile_pool(name="psB", bufs=3, space="PSUM"))
    pst = st.enter_context(tc.tile_pool(name="pst", bufs=1, space="PSUM"))
    drin = st.enter_context(tc.tile_pool(name="drin", bufs=4, space="DRAM"))
    drout = st.enter_context(tc.tile_pool(name="drout", bufs=4, space="DRAM"))
    drst = st.enter_context(tc.tile_pool(name="drst", bufs=2, space="DRAM"))

    # ------------------------------------------------ persistent tiles
    xbuf = persist.tile([P, DT, SL], f32, name="xbuf")     # x^T (fp32)
    xbf = persist.tile([P, DT, SL], bf16, name="xbf")      # x^T (bf16 copy)
    qT = persist.tile([P, QT, SL], bf16, name="qT")
    kT = persist.tile([P, QT, SL], bf16, name="kT")
    vsb = persist.tile([P, KVT, HPC * (HD + 1)], bf16, name="vsb")
    attnT = persist.tile([P, QT, SL], bf16, name="attnT")
    ident = persist.tile([P, P], f32, name="ident")
    ones1 = persist.tile([P, P], f32, name="ones1")
    onesb = persist.tile([P, P], bf16, name="onesb")
    idxs = persist.tile([P, SL // 16], i16, name="idxs")

    make_identity(nc, ident[:])
    nc.vector.memset(ones1[:], 1.0)
    nc.vector.memset(onesb[:], 1.0)
    # ones columns of vsb (col HD of each head block), set once
    for h in range(HPC):
        nc.scalar.activation(vsb[:, :, h * (HD + 1) + HD:h * (HD + 1) + HD + 1],
                             ident[:, 0:KVT].unsqueeze(-1),
                             AF.Identity, bias=1.0, scale=0.0)
    for r_ in range(P // 16):
        nc.sync.dma_start(idxs[16 * r_:16 * (r_ + 1), :], io["idx"])

    # ---------------------------------------- embedding: x^T = pes^T + (emb[seq])^T
    nc.sync.dma_start(xbuf[:], io["pesT"].rearrange("(k p) s -> p k s", p=P))
    for half in range(KVT // 2):  # gather 256 tokens at a time
        gtile = gbuf.tile([P, 2, D], f32, tag="gt", name=f"gt{half}")
        nc.gpsimd.dma_gather(
            out_ap=gtile[:],
            in_ap=io["emb"],
            idxs_ap=idxs[:, half * 16:(half + 1) * 16],
            num_idxs=2 * P,
            num_idxs_reg=2 * P,
            elem_size=D,
            queue_num=0,
        )
        for j in range(2):
            t = half * 2 + j            # token tile index (= position tile)
            for k in range(DT):
                ptile = pst.tile([P, P], f32, tag="tp", name=f"tp{t}_{k}")
                nc.tensor.transpose(ptile[:], gtile[:, j, ts(k, P)], ident[:])
                nc.vector.tensor_tensor(
                    out=xbuf[:, k, ts(t, P)],
                    in0=ptile[:],
                    in1=xbuf[:, k, ts(t, P)],
                    op=ALU.add,
                )
    nc.vector.tensor_copy(xbf[:], xbuf[:])

    # ---------------------------------------- chunked partial-out -> pair AR
    def partial_to_ar(lbl, w_sb, nk, rhs_ch, ardt=bf16):
        """for c in 2 chunks: out[m] = sum_kt w[kt,m]^T @ rhs(kt, chunk c);
        DMA to DRAM, pair-AllReduce the chunk."""
        ar_outs = []
        for c in range(2):
            arin = drin.tile([D, CH2], ardt, tag=f"ari{ardt != bf16}",
                             name=f"ari{lbl}_{c}")
            arout = drout.tile([D, CH2], ardt, tag=f"aro{ardt != bf16}",
                               name=f"aro{lbl}_{c}")
            for m in range(DT):
                ps2 = psB.tile([P, CH2], f32, tag="mmh", name=f"o{lbl}_{c}_{m}")
                for kt in range(nk):
                    nc.tensor.matmul(ps2[:], w_sb[:, kt, ts(m, P)],
                                     rhs_ch(kt, c),
                                     start=(kt == 0), stop=(kt == nk - 1))
                if ardt == bf16:
                    osb = epool.tile([P, CH2], bf16, tag="eh",
                                     name=f"ob{lbl}_{c}_{m}")[:]
                else:
                    osb = efpool.tile([P, SL], f32, tag="ef",
                                      name=f"ob{lbl}_{c}_{m}")[:, 0:CH2]
                nc.vector.tensor_copy(osb, ps2[:])
                nc.sync.dma_start(arin[ts(m, P), :], osb)
            nc.gpsimd.collective_compute(
                "AllReduce", ALU.add, replica_groups=PAIRS,
                ins=[arin.opt()], outs=[arout.opt()])
            ar_outs.append(arout)
        return ar_outs

    # ---------------------------------------- batchnorm (chunked stats)
    def batchnorm(lbl, ar_outs, g_sb, be_sb):
        sts = []
        for c, arout in enumerate(ar_outs):
            ydt = arout.tensor.dtype
            if ydt == bf16:
                yt = ybuf.tile([P, DT, CH2], bf16, tag="yt", name=f"yt{lbl}{c}")
            else:
                yt = gbuf.tile([P, DT, CH2], f32, tag="ytf", name=f"yt{lbl}{c}")
            nc.sync.dma_start(yt[:], arout.rearrange("(k p) t -> p k t", p=P))
            xs = xbuf[:, :, c * CH2:(c + 1) * CH2]
            nc.vector.tensor_tensor(out=xs, in0=xs, in1=yt[:], op=ALU.add)
            stc = small.tile([P, 2 * DT], f32, tag=f"stc{c}", name=f"stc{lbl}{c}")
            nc.vector.reduce_sum(out=stc[:, 0:DT].unsqueeze(-1), in_=xs,
                                 axis=mybir.AxisListType.X)
            for k in range(DT):
                scr = epool.tile([P, CH2], bf16, tag="eh", name=f"sq{lbl}{c}_{k}")
                nc.scalar.activation(scr[:], xbuf[:, k, c * CH2:(c + 1) * CH2],
                                     AF.Square,
                                     accum_out=stc[:, DT + k:DT + k + 1])
            sts.append(stc)
        stats = small.tile([P, 2 * DT], f32, tag="st", name=f"st{lbl}")
        nc.vector.tensor_tensor(out=stats[:], in0=sts[0][:], in1=sts[1][:],
                                op=ALU.add)
        sin = drst.tile([P, 2 * DT], f32, tag="si", name=f"si{lbl}")
        sout = drst.tile([P, 2 * DT], f32, tag="so", addr_space="Shared",
                         name=f"so{lbl}")
        nc.sync.dma_start(sin, stats[:])
        nc.gpsimd.collective_compute(
            "AllReduce", ALU.add, replica_groups=ALL8,
            ins=[sin.opt()], outs=[sout.opt()])
        gstats = small.tile([P, 2 * DT], f32, tag="gs", name=f"gs{lbl}")
        nc.sync.dma_start(gstats[:], sout)
        # finalize: mean/var over 2T (each sample contributed twice)
        mean = small.tile([P, DT], f32, tag="mean", name=f"mean{lbl}")
        nc.vector.tensor_scalar_mul(mean[:], gstats[:, 0:DT], 1.0 / (2 * T))
        msq = small.tile([P, DT], f32, tag="msq", name=f"msq{lbl}")
        nc.vector.tensor_tensor(out=msq[:], in0=mean[:], in1=mean[:], op=ALU.mult)
        veps = small.tile([P, DT], f32, tag="veps", name=f"veps{lbl}")
        nc.vector.scalar_tensor_tensor(out=veps[:], in0=gstats[:, DT:2 * DT],
                                       scalar=1.0 / (2 * T),
                                       in1=msq[:], op0=ALU.mult, op1=ALU.subtract)
        nc.vector.tensor_scalar_add(veps[:], veps[:], EPS)
        rec = small.tile([P, DT], f32, tag="rec", name=f"rec{lbl}")
        nc.vector.reciprocal(rec[:], veps[:])
        rstd = small.tile([P, DT], f32, tag="rstd", name=f"rstd{lbl}")
        nc.scalar.sqrt(rstd[:], rec[:])
        sc = small.tile([P, DT], f32, tag="sc", name=f"sc{lbl}")
        nc.vector.tensor_tensor(out=sc[:], in0=g_sb[:], in1=rstd[:], op=ALU.mult)
        sh = small.tile([P, DT], f32, tag="sh", name=f"sh{lbl}")
        nc.vector.tensor_tensor(out=sh[:], in0=mean[:], in1=sc[:], op=ALU.mult)
        nc.vector.tensor_tensor(out=sh[:], in0=be_sb[:], in1=sh[:], op=ALU.subtract)
        for k in range(DT):
            nc.scalar.activation(xbuf[:, k, :], xbuf[:, k, :], AF.Identity,
                                 bias=sh[:, k:k + 1], scale=sc[:, k:k + 1])
        nc.vector.tensor_copy(xbf[:], xbuf[:])

    # ---------------------------------------- layers
    for l in range(n_layers):
        # ---- layer weights/params to SBUF (bf16)
        wq_sb = wqkv.tile([P, DT, DSH], bf16, tag="wq", name=f"wq{l}")
        wk_sb = wqkv.tile([P, DT, DSH], bf16, tag="wk", name=f"wk{l}")
        wv_sb = wqkv.tile([P, DT, DSH], bf16, tag="wv", name=f"wv{l}")
        wo_sb = wqkv.tile([P, QT, D], bf16, tag="wo", name=f"wo{l}")
        w1_sb = wff.tile([P, DT, FSH], bf16, tag="w1", name=f"w1{l}")
        w2_sb = wff.tile([P, FMT, D], bf16, tag="w2", name=f"w2{l}")
        nc.sync.dma_start(wq_sb[:], io["wq"][l].rearrange("(k p) m -> p k m", p=P))
        nc.sync.dma_start(wk_sb[:], io["wk"][l].rearrange("(k p) m -> p k m", p=P))
        nc.sync.dma_start(wv_sb[:], io["wv"][l].rearrange("(k p) m -> p k m", p=P))
        nc.sync.dma_start(wo_sb[:], io["wo"][l].rearrange("(k p) m -> p k m", p=P))
        nc.sync.dma_start(w1_sb[:], io["w1"][l].rearrange("(k p) m -> p k m", p=P))
        nc.sync.dma_start(w2_sb[:], io["w2"][l].rearrange("(k p) m -> p k m", p=P))

        bq_sb = small.tile([P, QT], f32, tag="bq", name=f"bq{l}")
        bk_sb = small.tile([P, QT], f32, tag="bk", name=f"bk{l}")
        bvr = small.tile([1, DSH], bf16, tag="bvr", name=f"bvr{l}")
        bvrf = small.tile([1, DSH], f32, tag="bvrf", name=f"bvrf{l}")
        b1_sb = small.tile([P, FMT], f32, tag="b1", name=f"b1{l}")
        nc.sync.dma_start(bq_sb[:], io["bq"][l].rearrange("(m p) -> p m", p=P))
        nc.sync.dma_start(bk_sb[:], io["bk"][l].rearrange("(m p) -> p m", p=P))
        nc.sync.dma_start(bvrf[:], io["bv"][l].rearrange("(o m) -> o m", o=1))
        nc.vector.tensor_copy(bvr[:], bvrf[:])
        nc.sync.dma_start(b1_sb[:], io["b1"][l].rearrange("(m p) -> p m", p=P))

        g1_sb = small.tile([P, DT], f32, tag="g1", name=f"g1{l}")
        be1_sb = small.tile([P, DT], f32, tag="be1", name=f"be1{l}")
        g2_sb = small.tile([P, DT], f32, tag="g2", name=f"g2{l}")
        be2_sb = small.tile([P, DT], f32, tag="be2", name=f"be2{l}")
        nc.sync.dma_start(g1_sb[:], io["g1"][l].rearrange("(k p) -> p k", p=P))
        nc.sync.dma_start(be1_sb[:], io["be1"][l].rearrange("(k p) -> p k", p=P))
        nc.sync.dma_start(g2_sb[:], io["g2"][l].rearrange("(k p) -> p k", p=P))
        nc.sync.dma_start(be2_sb[:], io["be2"][l].rearrange("(k p) -> p k", p=P))

        # ---- Q,K projections: [P, QT, SL] = W^T @ x^T (+bias, bf16 out)
        for m in range(QT):
            for nm, src, dst, b_sb in (("q", wq_sb, qT, bq_sb),
                                       ("k", wk_sb, kT, bk_sb)):
                psq = psA.tile([P, SL], f32, tag="mm", name=f"ps{nm}{l}_{m}")
                for k in range(DT):
                    nc.tensor.matmul(psq[:], src[:, k, ts(m, P)], xbf[:, k, :],
                                     start=(k == 0), stop=(k == DT - 1))
                nc.scalar.activation(dst[:, m, :], psq[:], AF.Identity,
                                     bias=b_sb[:, m:m + 1])

        # ---- V directly in token-major layout: V[tok, ch] = x @ Wv + bv
        for tt in range(KVT):
            psv = psA.tile([P, SL], f32, tag="mm", name=f"psv{l}_{tt}")
            nc.tensor.matmul(psv[:, 0:DSH], onesb[0:1, 0:P], bvr[:],
                             start=True, stop=False)
            for k in range(DT):
                nc.tensor.matmul(psv[:, 0:DSH], xbf[:, k, ts(tt, P)],
                                 wv_sb[:, k, :],
                                 start=False, stop=(k == DT - 1))
            nc.vector.tensor_copy(
                vsb[:, tt, :].rearrange("p (h x) -> p h x", x=HD + 1)
                [:, :, 0:HD],
                psv[:, 0:DSH].rearrange("p (h x) -> p h x", x=HD))

        # ---- attention per head
        for h in range(HPC):
            qt_, prow = h // 2, (h % 2) * HD
            vof = h * (HD + 1)
            ets = []
            for kvt in range(KVT):
                pss = psA.tile([P, SL], f32, tag="mm", name=f"pss{l}_{h}_{kvt}")
                nc.tensor.matmul(
                    pss[:],
                    kT[prow:prow + HD, qt_, ts(kvt, P)],
                    qT[prow:prow + HD, qt_, :],
                    start=True, stop=True)
                et = epool.tile([P, SL], bf16, tag="e", name=f"et{l}_{h}_{kvt}")
                nc.scalar.activation(et[:], pss[:], AF.Exp, scale=att_scale)
                ets.append(et)
            psu = psA.tile([P, SL], f32, tag="mm", name=f"psu{l}_{h}")
            for kvt in range(KVT):
                nc.tensor.matmul(psu[0:HD + 1, :],
                                 vsb[:, kvt, vof:vof + HD + 1],
                                 ets[kvt][:],
                                 start=(kvt == 0), stop=(kvt == KVT - 1))
            usb = efpool.tile([P, SL], f32, tag="ef", name=f"usb{l}_{h}")
            nc.scalar.copy(usb[0:HD + 1, :], psu[0:HD + 1, :])
            rsb = efpool.tile([P, SL], f32, tag="ef", name=f"rsb{l}_{h}")
            nc.vector.reciprocal(rsb[HD:HD + 1, :], usb[HD:HD + 1, :])
            psr = psA.tile([P, SL], f32, tag="mm", name=f"psr{l}_{h}")
            nc.tensor.matmul(psr[0:HD, :], ones1[HD:HD + 1, 0:HD],
                             rsb[HD:HD + 1, :], start=True, stop=True)
            nc.vector.tensor_tensor(out=attnT[prow:prow + HD, qt_, :],
                                    in0=usb[0:HD, :],
                                    in1=psr[0:HD, :], op=ALU.mult)

        # ---- Wo partial -> chunked pair AR -> BN1
        ar1 = partial_to_ar(
            f"o{l}", wo_sb, QT,
            lambda kt, c: attnT[:, kt, c * CH2:(c + 1) * CH2],
            ardt=bf16)
        batchnorm(f"a{l}", ar1, g1_sb, be1_sb)

        # ---- FFN
        ht = hpool.tile([P, FMT, SL], bf16, tag="ht", name=f"ht{l}")
        for m in range(FMT):
            ps1 = psA.tile([P, SL], f32, tag="mm", name=f"ps1{l}_{m}")
            for k in range(DT):
                nc.tensor.matmul(ps1[:], w1_sb[:, k, ts(m, P)], xbf[:, k, :],
                                 start=(k == 0), stop=(k == DT - 1))
            nc.scalar.activation(ht[:, m, :], ps1[:], AF.Relu,
                                 bias=b1_sb[:, m:m + 1])
        ar2 = partial_to_ar(
            f"f{l}", w2_sb, FMT,
            lambda kt, c: ht[:, kt, c * CH2:(c + 1) * CH2])
        batchnorm(f"f{l}", ar2, g2_sb, be2_sb)

    # ---------------------------------------- output x^T -> [D, SL] (fp32)
    nc.sync.dma_start(io["out"].rearrange("(k p) t -> p k t", p=P), xbuf[:])
    st.close()


# ================================================================ host side

def _bf(a):
    import ml_dtypes
    return np.ascontiguousarray(np.asarray(a, dtype=np.float32)
                                .astype(ml_dtypes.bfloat16))


def make_in_maps(inputs):
    f = lambda a: np.ascontiguousarray(np.asarray(a), dtype=np.float32)
    seq = np.asarray(inputs["sequence"]).astype(np.int16)       # [B, S]
    emb = f(inputs["emb"])
    pesT = np.ascontiguousarray(np.asarray(inputs["pes"], dtype=np.float32).T)
    Wq, Wk, Wv = (np.asarray(inputs[k]) for k in ("Wq", "Wk", "Wv"))
    Wo, W1, W2 = (np.asarray(inputs[k]) for k in ("Wo", "W1", "W2"))
    bq, bk, bv = f(inputs["bq"]), f(inputs["bk"]), f(inputs["bv"])
    b1 = f(inputs["b1"])
    g1, be1 = f(inputs["g1"]), f(inputs["be1"])
    g2, be2 = f(inputs["g2"]), f(inputs["be2"])

    in_maps = []
    for c in range(NC):
        b, t = c // TP, c % TP
        ds_ = slice(t * DSH, (t + 1) * DSH)
        fs_ = slice(t * FSH, (t + 1) * FSH)
        idx = np.ascontiguousarray(seq[b].reshape(SL // 16, 16).T)  # [16, 32]
        in_maps.append({
            "emb": emb,
            "idx": idx,
            "pesT": pesT,
            "wq": _bf(Wq[:, :, ds_]),
            "wk": _bf(Wk[:, :, ds_]),
            "wv": _bf(Wv[:, :, ds_]),
            "wo": _bf(Wo[:, ds_, :]),
            "w1": _bf(W1[:, :, fs_]),
            "w2": _bf(W2[:, fs_, :]),
            "bq": np.ascontiguousarray(bq[:, ds_]),
            "bk": np.ascontiguousarray(bk[:, ds_]),
            "bv": np.ascontiguousarray(bv[:, ds_]),
            "b1": np.ascontiguousarray(b1[:, fs_]),
            "g1": g1, "be1": be1, "g2": g2, "be2": be2,
        })
    return in_maps


def assemble(results):
    """[B,S,D] fp32 from per-core [D,SL] outs (cores 0,2,4,6)."""
    outs = []
    for b in range(B):
        o = np.asarray(results[TP * b]["out"]).astype(np.float32)  # [D, SL]
        outs.append(np.ascontiguousarray(o.T))                     # [SL, D]
    return np.stack(outs, axis=0)


_CACHE = {}


def _get_module():
    if "nc" not in _CACHE:
        _CACHE["nc"] = build_module()
    return _CACHE["nc"]


def kernel(**inputs):
    from concourse import bass_utils
    nc = _get_module()
    in_maps = make_in_maps(inputs)
    res = bass_utils.run_bass_kernel_spmd(nc, in_maps, list(range(NC)))
    return assemble(res.results)


# revision 15
# speedup vs baseline: 1.0423x; 1.0423x over previous
"""Trainium2 Bass kernel for a 6-layer post-BatchNorm transformer encoder.

Reference model:
  x = emb[seq] + pes                                  # [B,S,D] = [4,512,1024]
  6x: x = BN(x + attn(x)); x = BN(x + ffn(x))
  BN = per-channel batch stats over (B,S), eps=1e-3.

Sharding: dp=4 x tp=2 mesh over 8 NeuronCores. Core c owns sample b=c//2
(512 tokens) and tensor-parallel half t=c%2 (8 heads of QKV/Wo, 2048 of the
4096 FFN hidden units). Per sublayer the pair AllReduces its partial [D,512]
output in two half-token chunks (bf16, 0.5MB each) so the first chunk's
reduce overlaps the second chunk's matmuls, and the residual-add plus
BN-stat computation of chunk 0 overlaps the reduce of chunk 1. BatchNorm
batch statistics are combined with an 8KB 8-core AllReduce (each sample
counted twice -> divide by 2T).

x is kept in fp32 (residual adds, BN stats and apply are exact); a bf16 copy
feeds the PE. Weights are bf16 (host-converted). V is produced directly in
token-major layout by using x-tiles as the stationary operand (no PE
transposes); its bias comes from a ones-row rank-1 matmul. Attention per
head: scores^T = K_h @ Q_h^T, E = exp(scale*scores^T), U^T = V_h^T @ E^T
with softmax denominators accumulated via a ones-column appended to V,
inverted with the 1-instruction approx reciprocal and broadcast across
partitions by a rank-1 PE matmul.

Host side shards inputs per core and reassembles the 4 samples from cores
0,2,4,6 - no final gather collective.
"""

import os

import numpy as np

import concourse.bass as bass
import concourse.mybir as mybir
import concourse.tile as tile
from concourse import bacc
from concourse.bass import ts
from concourse.masks import make_identity

# ---------------------------------------------------------------- dims
V, D, L, H, B, S = 32000, 1024, 6, 16, 4, 512
HD = D // H            # 64
DF = 4 * D             # 4096
EPS = 1e-3
NC = 8                 # cores
P = 128                # partitions
T = B * S              # 2048 tokens total
SL = S                 # tokens per core (one sample)
CH2 = SL // 2          # AR chunk = 256 tokens
DT = D // P            # 8 d-tiles
TP = 2                 # tensor-parallel width
DSH = D // TP          # qkv out shard = 512
QT = DSH // P          # 4 q-tiles
HPC = H // TP          # heads per core = 8
FSH = DF // TP         # ffn hidden shard = 2048
FMT = FSH // P         # ffn1 m-tiles = 16
KVT = SL // P          # kv token tiles = 4

f32 = mybir.dt.float32
bf16 = mybir.dt.bfloat16
i16 = mybir.dt.int16
AF = mybir.ActivationFunctionType
ALU = mybir.AluOpType

PAIRS = [[0, 1], [2, 3], [4, 5], [6, 7]]
ALL8 = [list(range(NC))]

N_LAYERS = int(os.environ.get("TRN_KERNEL_LAYERS", str(L)))


def build_module(n_layers=None):
    if n_layers is None:
        n_layers = N_LAYERS
    nc = bacc.Bacc("TRN2", target_bir_lowering=False, debug=False,
                   num_devices=NC)

    dt_ = nc.dram_tensor
    io = {
        "emb": dt_("emb", [V, D], f32, kind="ExternalInput").ap(),
        "idx": dt_("idx", [16, SL // 16], i16, kind="ExternalInput").ap(),
        "pesT": dt_("pesT", [D, SL], f32, kind="ExternalInput").ap(),
        "wq": dt_("wq", [L, D, DSH], bf16, kind="ExternalInput").ap(),
        "wk": dt_("wk", [L, D, DSH], bf16, kind="ExternalInput").ap(),
        "wv": dt_("wv", [L, D, DSH], bf16, kind="ExternalInput").ap(),
        "wo": dt_("wo", [L, DSH, D], bf16, kind="ExternalInput").ap(),
        "w1": dt_("w1", [L, D, FSH], bf16, kind="ExternalInput").ap(),
        "w2": dt_("w2", [L, FSH, D], bf16, kind="ExternalInput").ap(),
        "bq": dt_("bq", [L, DSH], f32, kind="ExternalInput").ap(),
        "bk": dt_("bk", [L, DSH], f32, kind="ExternalInput").ap(),
        "bv": dt_("bv", [L, DSH], f32, kind="ExternalInput").ap(),
        "b1": dt_("b1", [L, FSH], f32, kind="ExternalInput").ap(),
        "g1": dt_("g1", [L, D], f32, kind="ExternalInput").ap(),
        "be1": dt_("be1", [L, D], f32, kind="ExternalInput").ap(),
        "g2": dt_("g2", [L, D], f32, kind="ExternalInput").ap(),
        "be2": dt_("be2", [L, D], f32, kind="ExternalInput").ap(),
        "out": dt_("out", [D, SL], f32, kind="ExternalOutput").ap(),
    }

    with tile.TileContext(nc) as tc:
        _build(tc, n_layers, io)
    nc.compile()
    return nc


def _build(tc, n_layers, io):
    from contextlib import ExitStack
    nc = tc.nc
    att_scale = 1.0 / np.sqrt(HD)

    # ------------------------------------------------ pools
    st = ExitStack()
    persist = st.enter_context(tc.tile_pool(name="persist", bufs=1))
    wqkv = st.enter_context(tc.tile_pool(name="wqkv", bufs=1))
    wff = st.enter_context(tc.tile_pool(name="wff", bufs=1))
    small = st.enter_context(tc.tile_pool(name="small", bufs=2))
    ybuf = st.enter_context(tc.tile_pool(name="ybuf", bufs=2))   # AR readback
    gbuf = st.enter_context(tc.tile_pool(name="gbuf", bufs=1))   # emb gather
    epool = st.enter_context(tc.tile_pool(name="epool", bufs=6))
    efpool = st.enter_context(tc.tile_pool(name="efpool", bufs=3))
    hpool = st.enter_context(tc.tile_pool(name="hpool", bufs=1))  # ffn hidden
    psA = st.enter_context(tc.tile_pool(name="psA", bufs=4, space="PSUM"))
    psB = st.enter_context(tc.tile_pool(name="psB", bufs=3, space="PSUM"))
    pst = st.enter_context(tc.tile_pool(name="pst", bufs=1, space="PSUM"))
    drin = st.enter_context(tc.tile_pool(name="drin", bufs=4, space="DRAM"))
    drout = st.enter_context(tc.tile_pool(name="drout", bufs=4, space="DRAM"))
    drst = st.enter_context(tc.tile_pool(name="drst", bufs=2, space="DRAM"))

    # ------------------------------------------------ persistent tiles
    xbuf = persist.tile([P, DT, SL], f32, name="xbuf")     # x^T (fp32)
    xbf = persist.tile([P, DT, SL], bf16, name="xbf")      # x^T (bf16 copy)
    qT = persist.tile([P, QT, SL], bf16, name="qT")
    kT = persist.tile([P, QT, SL], bf16, name="kT")
    vsb = persist.tile([P, KVT, HPC * (HD + 1)], bf16, name="vsb")
    attnT = persist.tile([P, QT, SL], bf16, name="attnT")
    ident = persist.tile([P, P], f32, name="ident")
    ones1 = persist.tile([P, P], f32, name="ones1")
    onesb = persist.tile([P, P], bf16, name="onesb")
    idxs = persist.tile([P, SL // 16], i16, name="idxs")

    make_identity(nc, ident[:])
    nc.vector.memset(ones1[:], 1.0)
    nc.vector.memset(onesb[:], 1.0)
    # ones columns of vsb (col HD of each head block), set once
    for h in range(HPC):
        nc.scalar.activation(vsb[:, :, h * (HD + 1) + HD:h * (HD + 1) + HD + 1],
                             ident[:, 0:KVT].unsqueeze(-1),
                             AF.Identity, bias=1.0, scale=0.0)
    for r_ in range(P // 16):
        nc.sync.dma_start(idxs[16 * r_:16 * (r_ + 1), :], io["idx"])

    # ---------------------------------------- embedding: x^T = pes^T + (emb[seq])^T
    nc.sync.dma_start(xbuf[:], io["pesT"].rearrange("(k p) s -> p k s", p=P))
    for half in range(KVT // 2):  # gather 256 tokens at a time
        gtile = gbuf.tile([P, 2, D], f32, tag="gt", name=f"gt{half}")
        nc.gpsimd.dma_gather(
            out_ap=gtile[:],
            in_ap=io["emb"],
            idxs_ap=idxs[:, half * 16:(half + 1) * 16],
            num_idxs=2 * P,
            num_idxs_reg=2 * P,
            elem_size=D,
            queue_num=0,
        )
        for j in range(2):
            t = half * 2 + j            # token tile index (= position tile)
            for k in range(DT):
                ptile = pst.tile([P, P], f32, tag="tp", name=f"tp{t}_{k}")
                nc.tensor.transpose(ptile[:], gtile[:, j, ts(k, P)], ident[:])
                nc.vector.tensor_tensor(
                    out=xbuf[:, k, ts(t, P)],
                    in0=ptile[:],
                    in1=xbuf[:, k, ts(t, P)],
                    op=ALU.add,
                )
    nc.vector.tensor_copy(xbf[:], xbuf[:])

    # ------------------- partial-out -> pair AR, chunked over channels
    MH = DT // 2  # m-tiles per AR chunk
    def partial_to_ar(lbl, w_sb, nk, rhs_ch, ardt=bf16):
        """chunk c covers out-channel tiles c*MH..c*MH+MH-1 (full tokens);
        the first chunk's AllReduce overlaps the second chunk's matmuls."""
        ar_outs = []
        for c in range(2):
            arin = drin.tile([D // 2, SL], ardt, tag=f"ari{ardt != bf16}",
                             name=f"ari{lbl}_{c}")
            arout = drout.tile([D // 2, SL], ardt, tag=f"aro{ardt != bf16}",
                               name=f"aro{lbl}_{c}")
            for mi in range(MH):
                m = c * MH + mi
                ps2 = psB.tile([P, SL], f32, tag="mmB", name=f"o{lbl}_{c}_{mi}")
                for kt in range(nk):
                    nc.tensor.matmul(ps2[:], w_sb[:, kt, ts(m, P)],
                                     rhs_ch(kt),
                                     start=(kt == 0), stop=(kt == nk - 1))
                if ardt == bf16:
                    osb = epool.tile([P, SL], bf16, tag="e",
                                     name=f"ob{lbl}_{c}_{mi}")
                else:
                    osb = efpool.tile([P, SL], f32, tag="ef",
                                      name=f"ob{lbl}_{c}_{mi}")
                nc.vector.tensor_copy(osb[:], ps2[:])
                nc.sync.dma_start(arin[ts(mi, P), :], osb[:])
            nc.gpsimd.collective_compute(
                "AllReduce", ALU.add, replica_groups=PAIRS,
                ins=[arin.opt()], outs=[arout.opt()])
            ar_outs.append(arout)
        return ar_outs

    # ---------------------------------------- batchnorm (channel chunks)
    def batchnorm(lbl, ar_outs, g_sb, be_sb):
        stats = small.tile([P, 2 * DT], f32, tag="st", name=f"st{lbl}")
        for c, arout in enumerate(ar_outs):
            ydt = arout.tensor.dtype
            if ydt == bf16:
                yt = ybuf.tile([P, MH, SL], bf16, tag="yt", name=f"yt{lbl}{c}")
            else:
                yt = gbuf.tile([P, MH, SL], f32, tag="ytf", name=f"yt{lbl}{c}")
            nc.sync.dma_start(yt[:], arout.rearrange("(k p) t -> p k t", p=P))
            xs = xbuf[:, c * MH:(c + 1) * MH, :]
            nc.vector.tensor_tensor(out=xs, in0=xs, in1=yt[:], op=ALU.add)
            nc.vector.reduce_sum(
                out=stats[:, c * MH:(c + 1) * MH].unsqueeze(-1), in_=xs,
                axis=mybir.AxisListType.X)
            for ki in range(MH):
                k = c * MH + ki
                scr = epool.tile([P, SL], bf16, tag="e", name=f"sq{lbl}{c}_{ki}")
                nc.scalar.activation(scr[:], xbuf[:, k, :], AF.Square,
                                     accum_out=stats[:, DT + k:DT + k + 1])
        sin = drst.tile([P, 2 * DT], f32, tag="si", name=f"si{lbl}")
        sout = drst.tile([P, 2 * DT], f32, tag="so", addr_space="Shared",
                         name=f"so{lbl}")
        nc.sync.dma_start(sin, stats[:])
        nc.gpsimd.collective_compute(
            "AllReduce", ALU.add, replica_groups=ALL8,
            ins=[sin.opt()], outs=[sout.opt()])
        gstats = small.tile([P, 2 * DT], f32, tag="gs", name=f"gs{lbl}")
        nc.sync.dma_start(gstats[:], sout)
        # finalize: mean/var over 2T (each sample contributed twice)
        mean = small.tile([P, DT], f32, tag="mean", name=f"mean{lbl}")
        nc.vector.tensor_scalar_mul(mean[:], gstats[:, 0:DT], 1.0 / (2 * T))
        msq = small.tile([P, DT], f32, tag="msq", name=f"msq{lbl}")
        nc.vector.tensor_tensor(out=msq[:], in0=mean[:], in1=mean[:], op=ALU.mult)
        veps = small.tile([P, DT], f32, tag="veps", name=f"veps{lbl}")
        nc.vector.scalar_tensor_tensor(out=veps[:], in0=gstats[:, DT:2 * DT],
                                       scalar=1.0 / (2 * T),
                                       in1=msq[:], op0=ALU.mult, op1=ALU.subtract)
        nc.vector.tensor_scalar_add(veps[:], veps[:], EPS)
        rec = small.tile([P, DT], f32, tag="rec", name=f"rec{lbl}")
        nc.vector.reciprocal(rec[:], veps[:])
        rstd = small.tile([P, DT], f32, tag="rstd", name=f"rstd{lbl}")
        nc.scalar.sqrt(rstd[:], rec[:])
        sc = small.tile([P, DT], f32, tag="sc", name=f"sc{lbl}")
        nc.vector.tensor_tensor(out=sc[:], in0=g_sb[:], in1=rstd[:], op=ALU.mult)
        sh = small.tile([P, DT], f32, tag="sh", name=f"sh{lbl}")
        nc.vector.tensor_tensor(out=sh[:], in0=mean[:], in1=sc[:], op=ALU.mult)
        nc.vector.tensor_tensor(out=sh[:], in0=be_sb[:], in1=sh[:], op=ALU.subtract)
        for k in range(DT):
            nc.scalar.activation(xbuf[:, k, :], xbuf[:, k, :], AF.Identity,
                                 bias=sh[:, k:k + 1], scale=sc[:, k:k + 1])
            nc.vector.tensor_copy(xbf[:, k, :], xbuf[:, k, :])

    # ---------------------------------------- weight/param loaders
    def load_qk(l):
        if l >= n_layers:
            return None
        wq_sb = wqkv.tile([P, DT, DSH], bf16, tag="wq", name=f"wq{l}")
        wk_sb = wqkv.tile([P, DT, DSH], bf16, tag="wk", name=f"wk{l}")
        nc.sync.dma_start(wq_sb[:], io["wq"][l].rearrange("(k p) m -> p k m", p=P))
        nc.sync.dma_start(wk_sb[:], io["wk"][l].rearrange("(k p) m -> p k m", p=P))
        return wq_sb, wk_sb

    def load_wv(l):
        if l >= n_layers:
            return None
        wv_sb = wqkv.tile([P, DT, DSH], bf16, tag="wv", name=f"wv{l}")
        nc.sync.dma_start(wv_sb[:], io["wv"][l].rearrange("(k p) m -> p k m", p=P))
        return wv_sb

    def load_wo(l):
        if l >= n_layers:
            return None
        wo_sb = wqkv.tile([P, QT, D], bf16, tag="wo", name=f"wo{l}")
        nc.sync.dma_start(wo_sb[:], io["wo"][l].rearrange("(k p) m -> p k m", p=P))
        return wo_sb

    def load_w1(l):
        if l >= n_layers:
            return None
        w1_sb = wff.tile([P, DT, FSH], bf16, tag="w1", name=f"w1{l}")
        nc.sync.dma_start(w1_sb[:], io["w1"][l].rearrange("(k p) m -> p k m", p=P))
        return w1_sb

    def load_w2(l):
        if l >= n_layers:
            return None
        w2_sb = wff.tile([P, FMT, D], bf16, tag="w2", name=f"w2{l}")
        nc.sync.dma_start(w2_sb[:], io["w2"][l].rearrange("(k p) m -> p k m", p=P))
        return w2_sb

    def load_params(l):
        if l >= n_layers:
            return None
        p = {}
        p["bq"] = small.tile([P, QT], f32, tag="bq", name=f"bq{l}")
        p["bk"] = small.tile([P, QT], f32, tag="bk", name=f"bk{l}")
        bvr = small.tile([1, DSH], bf16, tag="bvr", name=f"bvr{l}")
        bvrf = small.tile([1, DSH], f32, tag="bvrf", name=f"bvrf{l}")
        p["b1"] = small.tile([P, FMT], f32, tag="b1", name=f"b1{l}")
        nc.sync.dma_start(p["bq"][:], io["bq"][l].rearrange("(m p) -> p m", p=P))
        nc.sync.dma_start(p["bk"][:], io["bk"][l].rearrange("(m p) -> p m", p=P))
        nc.sync.dma_start(bvrf[:], io["bv"][l].rearrange("(o m) -> o m", o=1))
        nc.vector.tensor_copy(bvr[:], bvrf[:])
        nc.sync.dma_start(p["b1"][:], io["b1"][l].rearrange("(m p) -> p m", p=P))
        p["bv"] = bvr
        for nm in ("g1", "be1", "g2", "be2"):
            p[nm] = small.tile([P, DT], f32, tag=nm, name=f"{nm}{l}")
            nc.sync.dma_start(p[nm][:], io[nm][l].rearrange("(k p) -> p k", p=P))
        return p

    # layer-0 loads issued after the embedding instructions so the gather and
    # pes DMAs win queue order; they complete during the transpose phase.
    cur = (load_qk(0), load_wv(0), load_wo(0), load_w1(0), load_w2(0),
           load_params(0))

    # ---------------------------------------- layers
    for l in range(n_layers):
        (wq_sb, wk_sb), wv_sb, wo_sb, w1_sb, w2_sb, prm = cur
        bq_sb, bk_sb, bvr, b1_sb = prm["bq"], prm["bk"], prm["bv"], prm["b1"]
        g1_sb, be1_sb, g2_sb, be2_sb = (prm["g1"], prm["be1"], prm["g2"],
                                        prm["be2"])

        # ---- Q,K projections: [P, QT, SL] = W^T @ x^T (+bias, bf16 out)
        for m in range(QT):
            for nm, src, dst, b_sb in (("q", wq_sb, qT, bq_sb),
                                       ("k", wk_sb, kT, bk_sb)):
                psq = psA.tile([P, SL], f32, tag="mm", name=f"ps{nm}{l}_{m}")
                for k in range(DT):
                    nc.tensor.matmul(psq[:], src[:, k, ts(m, P)], xbf[:, k, :],
                                     start=(k == 0), stop=(k == DT - 1))
                nc.scalar.activation(dst[:, m, :], psq[:], AF.Identity,
                                     bias=b_sb[:, m:m + 1])
        nxt_qk = load_qk(l + 1)

        # ---- V directly in token-major layout: V[tok, ch] = x @ Wv + bv
        for tt in range(KVT):
            psv = psA.tile([P, SL], f32, tag="mm", name=f"psv{l}_{tt}")
            nc.tensor.matmul(psv[:, 0:DSH], onesb[0:1, 0:P], bvr[:],
                             start=True, stop=False)
            for k in range(DT):
                nc.tensor.matmul(psv[:, 0:DSH], xbf[:, k, ts(tt, P)],
                                 wv_sb[:, k, :],
                                 start=False, stop=(k == DT - 1))
            nc.vector.tensor_copy(
                vsb[:, tt, :].rearrange("p (h x) -> p h x", x=HD + 1)
                [:, :, 0:HD],
                psv[:, 0:DSH].rearrange("p (h x) -> p h x", x=HD))
        nxt_wv = load_wv(l + 1)

        # ---- attention per head
        for h in range(HPC):
            qt_, prow = h // 2, (h % 2) * HD
            vof = h * (HD + 1)
            ets = []
            for kvt in range(KVT):
                pss = psA.tile([P, SL], f32, tag="mm", name=f"pss{l}_{h}_{kvt}")
                nc.tensor.matmul(
                    pss[:],
                    kT[prow:prow + HD, qt_, ts(kvt, P)],
                    qT[prow:prow + HD, qt_, :],
                    start=True, stop=True)
                et = epool.tile([P, SL], bf16, tag="e", name=f"et{l}_{h}_{kvt}")
                nc.scalar.activation(et[:], pss[:], AF.Exp, scale=att_scale)
                ets.append(et)
            psu = psA.tile([P, SL], f32, tag="mm", name=f"psu{l}_{h}")
            for kvt in range(KVT):
                nc.tensor.matmul(psu[0:HD + 1, :],
                                 vsb[:, kvt, vof:vof + HD + 1],
                                 ets[kvt][:],
                                 start=(kvt == 0), stop=(kvt == KVT - 1))
            usb = efpool.tile([P, SL], f32, tag="ef", name=f"usb{l}_{h}")
            nc.scalar.copy(usb[0:HD + 1, :], psu[0:HD + 1, :])
            rsb = efpool.tile([P, SL], f32, tag="ef", name=f"rsb{l}_{h}")
            nc.vector.reciprocal(rsb[HD:HD + 1, :], usb[HD:HD + 1, :])
            psr = psA.tile([P, SL], f32, tag="mm", name=f"psr{l}_{h}")
            nc.tensor.matmul(psr[0:HD, :], ones1[HD:HD + 1, 0:HD],
                             rsb[HD:HD + 1, :], start=True, stop=True)
            nc.vector.tensor_tensor(out=attnT[prow:prow + HD, qt_, :],
                                    in0=usb[0:HD, :],
                                    in1=psr[0:HD, :], op=ALU.mult)

        nxt_prm = load_params(l + 1)

        # ---- Wo partial -> chunked pair AR -> BN1
        ar1 = partial_to_ar(f"o{l}", wo_sb, QT, lambda kt: attnT[:, kt, :],
                            ardt=bf16)
        nxt_wo = load_wo(l + 1)
        batchnorm(f"a{l}", ar1, g1_sb, be1_sb)

        # ---- FFN
        ht = hpool.tile([P, FMT, SL], bf16, tag="ht", name=f"ht{l}")
        for m in range(FMT):
            ps1 = psA.tile([P, SL], f32, tag="mm", name=f"ps1{l}_{m}")
            for k in range(DT):
                nc.tensor.matmul(ps1[:], w1_sb[:, k, ts(m, P)], xbf[:, k, :],
                                 start=(k == 0), stop=(k == DT - 1))
            nc.scalar.activation(ht[:, m, :], ps1[:], AF.Relu,
                                 bias=b1_sb[:, m:m + 1])
        nxt_w1 = load_w1(l + 1)
        ar2 = partial_to_ar(f"f{l}", w2_sb, FMT, lambda kt: ht[:, kt, :])
        nxt_w2 = load_w2(l + 1)
        batchnorm(f"f{l}", ar2, g2_sb, be2_sb)
        cur = (nxt_qk, nxt_wv, nxt_wo, nxt_w1, nxt_w2, nxt_prm)

    # ---------------------------------------- output x^T -> [D, SL] (fp32)
    nc.sync.dma_start(io["out"].rearrange("(k p) t -> p k t", p=P), xbuf[:])
    st.close()


# ================================================================ host side

def _bf(a):
    import ml_dtypes
    return np.ascontiguousarray(np.asarray(a, dtype=np.float32)
                                .astype(ml_dtypes.bfloat16))


def make_in_maps(inputs):
    f = lambda a: np.ascontiguousarray(np.asarray(a), dtype=np.float32)
    seq = np.asarray(inputs["sequence"]).astype(np.int16)       # [B, S]
    emb = f(inputs["emb"])
    pesT = np.ascontiguousarray(np.asarray(inputs["pes"], dtype=np.float32).T)
    Wq, Wk, Wv = (np.asarray(inputs[k]) for k in ("Wq", "Wk", "Wv"))
    Wo, W1, W2 = (np.asarray(inputs[k]) for k in ("Wo", "W1", "W2"))
    bq, bk, bv = f(inputs["bq"]), f(inputs["bk"]), f(inputs["bv"])
    b1 = f(inputs["b1"])
    g1, be1 = f(inputs["g1"]), f(inputs["be1"])
    g2, be2 = f(inputs["g2"]), f(inputs["be2"])

    in_maps = []
    for c in range(NC):
        b, t = c // TP, c % TP
        ds_ = slice(t * DSH, (t + 1) * DSH)
        fs_ = slice(t * FSH, (t + 1) * FSH)
        idx = np.ascontiguousarray(seq[b].reshape(SL // 16, 16).T)  # [16, 32]
        in_maps.append({
            "emb": emb,
            "idx": idx,
            "pesT": pesT,
            "wq": _bf(Wq[:, :, ds_]),
            "wk": _bf(Wk[:, :, ds_]),
            "wv": _bf(Wv[:, :, ds_]),
            "wo": _bf(Wo[:, ds_, :]),
            "w1": _bf(W1[:, :, fs_]),
            "w2": _bf(W2[:, fs_, :]),
            "bq": np.ascontiguousarray(bq[:, ds_]),
            "bk": np.ascontiguousarray(bk[:, ds_]),
            "bv": np.ascontiguousarray(bv[:, ds_]),
            "b1": np.ascontiguousarray(b1[:, fs_]),
            "g1": g1, "be1": be1, "g2": g2, "be2": be2,
        })
    return in_maps


def assemble(results):
    """[B,S,D] fp32 from per-core [D,SL] outs (cores 0,2,4,6)."""
    outs = []
    for b in range(B):
        o = np.asarray(results[TP * b]["out"]).astype(np.float32)  # [D, SL]
        outs.append(np.ascontiguousarray(o.T))                     # [SL, D]
    return np.stack(outs, axis=0)


_CACHE = {}


def _get_module():
    if "nc" not in _CACHE:
        _CACHE["nc"] = build_module()
    return _CACHE["nc"]


def kernel(**inputs):
    from concourse import bass_utils
    nc = _get_module()
    in_maps = make_in_maps(inputs)
    res = bass_utils.run_bass_kernel_spmd(nc, in_maps, list(range(NC)))
    return assemble(res.results)


# revision 17
# speedup vs baseline: 1.1503x; 1.1037x over previous
"""Trainium2 Bass kernel for a 6-layer post-BatchNorm transformer encoder.

Reference model:
  x = emb[seq] + pes                                  # [B,S,D] = [4,512,1024]
  6x: x = BN(x + attn(x)); x = BN(x + ffn(x))
  BN = per-channel batch stats over (B,S), eps=1e-3.

Sharding: dp=4 x tp=2 mesh over 8 NeuronCores. Core c owns sample b=c//2
(512 tokens) and tensor-parallel half t=c%2 (8 heads of QKV/Wo, 2048 of the
4096 FFN hidden units). Per sublayer the pair AllReduces its partial [D,512]
output in two half-token chunks (bf16, 0.5MB each) so the first chunk's
reduce overlaps the second chunk's matmuls, and the residual-add plus
BN-stat computation of chunk 0 overlaps the reduce of chunk 1. BatchNorm
batch statistics are combined with an 8KB 8-core AllReduce (each sample
counted twice -> divide by 2T).

x is kept in fp32 (residual adds, BN stats and apply are exact); a bf16 copy
feeds the PE. Weights are bf16 (host-converted). V is produced directly in
token-major layout by using x-tiles as the stationary operand (no PE
transposes); its bias comes from a ones-row rank-1 matmul. Attention per
head: scores^T = K_h @ Q_h^T, E = exp(scale*scores^T), U^T = V_h^T @ E^T
with softmax denominators accumulated via a ones-column appended to V,
inverted with the 1-instruction approx reciprocal and broadcast across
partitions by a rank-1 PE matmul.

Host side shards inputs per core and reassembles the 4 samples from cores
0,2,4,6 - no final gather collective.
"""

import os

import numpy as np

import concourse.bass as bass
import concourse.mybir as mybir
import concourse.tile as tile
from concourse import bacc
from concourse.bass import ts
from concourse.masks import make_identity

# ---------------------------------------------------------------- dims
V, D, L, H, B, S = 32000, 1024, 6, 16, 4, 512
HD = D // H            # 64
DF = 4 * D             # 4096
EPS = 1e-3
NC = 8                 # cores
P = 128                # partitions
T = B * S              # 2048 tokens total
SL = S                 # tokens per core (one sample)
CH2 = SL // 2          # AR chunk = 256 tokens
DT = D // P            # 8 d-tiles
TP = 2                 # tensor-parallel width
DSH = D // TP          # qkv out shard = 512
QT = DSH // P          # 4 q-tiles
HPC = H // TP          # heads per core = 8
FSH = DF // TP         # ffn hidden shard = 2048
FMT = FSH // P         # ffn1 m-tiles = 16
KVT = SL // P          # kv token tiles = 4

f32 = mybir.dt.float32
bf16 = mybir.dt.bfloat16
i16 = mybir.dt.int16
AF = mybir.ActivationFunctionType
ALU = mybir.AluOpType

PAIRS = [[0, 1], [2, 3], [4, 5], [6, 7]]
ALL8 = [list(range(NC))]

N_LAYERS = int(os.environ.get("TRN_KERNEL_LAYERS", str(L)))


def build_module(n_layers=None):
    if n_layers is None:
        n_layers = N_LAYERS
    nc = bacc.Bacc("TRN2", target_bir_lowering=False, debug=False,
                   num_devices=NC)

    dt_ = nc.dram_tensor
    io = {
        "emb": dt_("emb", [V, D], f32, kind="ExternalInput").ap(),
        "idx": dt_("idx", [16, SL // 16], i16, kind="ExternalInput").ap(),
        "pesT": dt_("pesT", [D, SL], f32, kind="ExternalInput").ap(),
        "wq": dt_("wq", [L, D, DSH], bf16, kind="ExternalInput").ap(),
        "wk": dt_("wk", [L, D, DSH], bf16, kind="ExternalInput").ap(),
        "wv": dt_("wv", [L, D, DSH], bf16, kind="ExternalInput").ap(),
        "wo": dt_("wo", [L, DSH, D], bf16, kind="ExternalInput").ap(),
        "w1": dt_("w1", [L, D, FSH], bf16, kind="ExternalInput").ap(),
        "w2": dt_("w2", [L, FSH, D], bf16, kind="ExternalInput").ap(),
        "bq": dt_("bq", [L, DSH], f32, kind="ExternalInput").ap(),
        "bk": dt_("bk", [L, DSH], f32, kind="ExternalInput").ap(),
        "bv": dt_("bv", [L, DSH], f32, kind="ExternalInput").ap(),
        "b1": dt_("b1", [L, FSH], f32, kind="ExternalInput").ap(),
        "g1": dt_("g1", [L, D], f32, kind="ExternalInput").ap(),
        "be1": dt_("be1", [L, D], f32, kind="ExternalInput").ap(),
        "g2": dt_("g2", [L, D], f32, kind="ExternalInput").ap(),
        "be2": dt_("be2", [L, D], f32, kind="ExternalInput").ap(),
        "out": dt_("out", [D, SL], f32, kind="ExternalOutput").ap(),
    }

    with tile.TileContext(nc) as tc:
        _build(tc, n_layers, io)
    nc.compile()
    return nc


def _build(tc, n_layers, io):
    from contextlib import ExitStack
    nc = tc.nc
    att_scale = 1.0 / np.sqrt(HD)

    # ------------------------------------------------ pools
    st = ExitStack()
    persist = st.enter_context(tc.tile_pool(name="persist", bufs=1))
    wqkv = st.enter_context(tc.tile_pool(name="wqkv", bufs=1))
    wff = st.enter_context(tc.tile_pool(name="wff", bufs=1))
    small = st.enter_context(tc.tile_pool(name="small", bufs=2))
    ybuf = st.enter_context(tc.tile_pool(name="ybuf", bufs=2))   # AR readback
    gbuf = st.enter_context(tc.tile_pool(name="gbuf", bufs=1))   # emb gather
    epool = st.enter_context(tc.tile_pool(name="epool", bufs=5))
    upool = st.enter_context(tc.tile_pool(name="upool", bufs=9))
    hpool = st.enter_context(tc.tile_pool(name="hpool", bufs=1))  # ffn hidden
    psA = st.enter_context(tc.tile_pool(name="psA", bufs=3, space="PSUM"))
    psS = st.enter_context(tc.tile_pool(name="psS", bufs=3, space="PSUM"))
    psUR = st.enter_context(tc.tile_pool(name="psUR", bufs=2, space="PSUM"))
    drin = st.enter_context(tc.tile_pool(name="drin", bufs=4, space="DRAM"))
    drout = st.enter_context(tc.tile_pool(name="drout", bufs=4, space="DRAM"))
    drst = st.enter_context(tc.tile_pool(name="drst", bufs=2, space="DRAM"))

    # ------------------------------------------------ persistent tiles
    xbuf = persist.tile([P, DT, SL], f32, name="xbuf")     # x^T (fp32)
    xbf = persist.tile([P, DT, SL], bf16, name="xbf")      # x^T (bf16 copy)
    qT = persist.tile([P, QT, SL], bf16, name="qT")
    kT = persist.tile([P, QT, SL], bf16, name="kT")
    vsb = persist.tile([P, KVT, HPC * (HD + 1)], bf16, name="vsb")
    attnT = persist.tile([P, QT, SL], bf16, name="attnT")
    ident = persist.tile([P, P], f32, name="ident")
    ones1 = persist.tile([P, P], f32, name="ones1")
    onesb = persist.tile([P, P], bf16, name="onesb")
    idxs = persist.tile([P, SL // 16], i16, name="idxs")

    make_identity(nc, ident[:])
    nc.vector.memset(ones1[:], 1.0)
    nc.vector.memset(onesb[:], 1.0)
    # ones columns of vsb (col HD of each head block), set once
    for h in range(HPC):
        nc.scalar.activation(vsb[:, :, h * (HD + 1) + HD:h * (HD + 1) + HD + 1],
                             ident[:, 0:KVT].unsqueeze(-1),
                             AF.Identity, bias=1.0, scale=0.0)
    for r_ in range(P // 16):
        nc.sync.dma_start(idxs[16 * r_:16 * (r_ + 1), :], io["idx"])

    # ---------------------------------------- embedding: x^T = pes^T + (emb[seq])^T
    nc.sync.dma_start(xbuf[:], io["pesT"].rearrange("(k p) s -> p k s", p=P))
    for half in range(KVT // 2):  # gather 256 tokens at a time
        gtile = gbuf.tile([P, 2, D], f32, tag="gt", name=f"gt{half}")
        nc.gpsimd.dma_gather(
            out_ap=gtile[:],
            in_ap=io["emb"],
            idxs_ap=idxs[:, half * 16:(half + 1) * 16],
            num_idxs=2 * P,
            num_idxs_reg=2 * P,
            elem_size=D,
            queue_num=0,
        )
        for j in range(2):
            t = half * 2 + j            # token tile index (= position tile)
            for k in range(DT):
                ptile = psA.tile([P, SL], f32, tag="mm", name=f"tp{t}_{k}")
                nc.tensor.transpose(ptile[:, 0:P], gtile[:, j, ts(k, P)],
                                    ident[:])
                nc.vector.tensor_tensor(
                    out=xbuf[:, k, ts(t, P)],
                    in0=ptile[:, 0:P],
                    in1=xbuf[:, k, ts(t, P)],
                    op=ALU.add,
                )
    nc.vector.tensor_copy(xbf[:], xbuf[:])

    # ------------------- partial-out -> pair AR, chunked over channels
    MH = DT // 2  # m-tiles per AR chunk
    def partial_to_ar(lbl, w_sb, nk, rhs_ch, ardt=bf16):
        """chunk c covers out-channel tiles c*MH..c*MH+MH-1 (full tokens);
        the first chunk's AllReduce overlaps the second chunk's matmuls."""
        ar_outs = []
        for c in range(2):
            arin = drin.tile([D // 2, SL], ardt, tag=f"ari{ardt != bf16}",
                             name=f"ari{lbl}_{c}")
            arout = drout.tile([D // 2, SL], ardt, tag=f"aro{ardt != bf16}",
                               name=f"aro{lbl}_{c}")
            for mi in range(MH):
                m = c * MH + mi
                ps2 = psA.tile([P, SL], f32, tag="mm", name=f"o{lbl}_{c}_{mi}")
                for kt in range(nk):
                    nc.tensor.matmul(ps2[:], w_sb[:, kt, ts(m, P)],
                                     rhs_ch(kt),
                                     start=(kt == 0), stop=(kt == nk - 1))
                osb = epool.tile([P, SL], ardt, tag="e",
                                 name=f"ob{lbl}_{c}_{mi}")
                nc.vector.tensor_copy(osb[:], ps2[:])
                nc.sync.dma_start(arin[ts(mi, P), :], osb[:])
            nc.gpsimd.collective_compute(
                "AllReduce", ALU.add, replica_groups=PAIRS,
                ins=[arin.opt()], outs=[arout.opt()])
            ar_outs.append(arout)
        return ar_outs

    # ---------------------------------------- batchnorm (channel chunks)
    def batchnorm(lbl, ar_outs, g_sb, be_sb):
        stats = small.tile([P, 2 * DT], f32, tag="st", name=f"st{lbl}")
        for c, arout in enumerate(ar_outs):
            ydt = arout.tensor.dtype
            if ydt == bf16:
                yt = ybuf.tile([P, MH, SL], bf16, tag="yt", name=f"yt{lbl}{c}")
            else:
                yt = gbuf.tile([P, MH, SL], f32, tag="ytf", name=f"yt{lbl}{c}")
            nc.sync.dma_start(yt[:], arout.rearrange("(k p) t -> p k t", p=P))
            xs = xbuf[:, c * MH:(c + 1) * MH, :]
            nc.vector.tensor_tensor(out=xs, in0=xs, in1=yt[:], op=ALU.add)
            nc.vector.reduce_sum(
                out=stats[:, c * MH:(c + 1) * MH].unsqueeze(-1), in_=xs,
                axis=mybir.AxisListType.X)
            for ki in range(MH):
                k = c * MH + ki
                scr = epool.tile([P, SL], bf16, tag="e", name=f"sq{lbl}{c}_{ki}")
                nc.scalar.activation(scr[:], xbuf[:, k, :], AF.Square,
                                     accum_out=stats[:, DT + k:DT + k + 1])
        sin = drst.tile([P, 2 * DT], f32, tag="si", name=f"si{lbl}")
        sout = drst.tile([P, 2 * DT], f32, tag="so", addr_space="Shared",
                         name=f"so{lbl}")
        nc.sync.dma_start(sin, stats[:])
        nc.gpsimd.collective_compute(
            "AllReduce", ALU.add, replica_groups=ALL8,
            ins=[sin.opt()], outs=[sout.opt()])
        gstats = small.tile([P, 2 * DT], f32, tag="gs", name=f"gs{lbl}")
        nc.sync.dma_start(gstats[:], sout)
        # finalize: mean/var over 2T (each sample contributed twice)
        mean = small.tile([P, DT], f32, tag="mean", name=f"mean{lbl}")
        nc.vector.tensor_scalar_mul(mean[:], gstats[:, 0:DT], 1.0 / (2 * T))
        msq = small.tile([P, DT], f32, tag="msq", name=f"msq{lbl}")
        nc.vector.tensor_tensor(out=msq[:], in0=mean[:], in1=mean[:], op=ALU.mult)
        veps = small.tile([P, DT], f32, tag="veps", name=f"veps{lbl}")
        nc.vector.scalar_tensor_tensor(out=veps[:], in0=gstats[:, DT:2 * DT],
                                       scalar=1.0 / (2 * T),
                                       in1=msq[:], op0=ALU.mult, op1=ALU.subtract)
        nc.vector.tensor_scalar_add(veps[:], veps[:], EPS)
        rec = small.tile([P, DT], f32, tag="rec", name=f"rec{lbl}")
        nc.vector.reciprocal(rec[:], veps[:])
        rstd = small.tile([P, DT], f32, tag="rstd", name=f"rstd{lbl}")
        nc.scalar.sqrt(rstd[:], rec[:])
        sc = small.tile([P, DT], f32, tag="sc", name=f"sc{lbl}")
        nc.vector.tensor_tensor(out=sc[:], in0=g_sb[:], in1=rstd[:], op=ALU.mult)
        sh = small.tile([P, DT], f32, tag="sh", name=f"sh{lbl}")
        nc.vector.tensor_tensor(out=sh[:], in0=mean[:], in1=sc[:], op=ALU.mult)
        nc.vector.tensor_tensor(out=sh[:], in0=be_sb[:], in1=sh[:], op=ALU.subtract)
        for k in range(DT):
            nc.scalar.activation(xbuf[:, k, :], xbuf[:, k, :], AF.Identity,
                                 bias=sh[:, k:k + 1], scale=sc[:, k:k + 1])
            nc.vector.tensor_copy(xbf[:, k, :], xbuf[:, k, :])

    # ---------------------------------------- weight/param loaders
    def load_qk(l):
        if l >= n_layers:
            return None
        wq_sb = wqkv.tile([P, DT, DSH], bf16, tag="wq", name=f"wq{l}")
        wk_sb = wqkv.tile([P, DT, DSH], bf16, tag="wk", name=f"wk{l}")
        nc.sync.dma_start(wq_sb[:], io["wq"][l].rearrange("(k p) m -> p k m", p=P))
        nc.sync.dma_start(wk_sb[:], io["wk"][l].rearrange("(k p) m -> p k m", p=P))
        return wq_sb, wk_sb

    def load_wv(l):
        if l >= n_layers:
            return None
        wv_sb = wqkv.tile([P, DT, DSH], bf16, tag="wv", name=f"wv{l}")
        nc.sync.dma_start(wv_sb[:], io["wv"][l].rearrange("(k p) m -> p k m", p=P))
        return wv_sb

    def load_wo(l):
        if l >= n_layers:
            return None
        wo_sb = wqkv.tile([P, QT, D], bf16, tag="wo", name=f"wo{l}")
        nc.sync.dma_start(wo_sb[:], io["wo"][l].rearrange("(k p) m -> p k m", p=P))
        return wo_sb

    def load_w1(l):
        if l >= n_layers:
            return None
        w1_sb = wff.tile([P, DT, FSH], bf16, tag="w1", name=f"w1{l}")
        nc.sync.dma_start(w1_sb[:], io["w1"][l].rearrange("(k p) m -> p k m", p=P))
        return w1_sb

    def load_w2(l):
        if l >= n_layers:
            return None
        w2_sb = wff.tile([P, FMT, D], bf16, tag="w2", name=f"w2{l}")
        nc.sync.dma_start(w2_sb[:], io["w2"][l].rearrange("(k p) m -> p k m", p=P))
        return w2_sb

    def load_params(l):
        if l >= n_layers:
            return None
        p = {}
        p["bq"] = small.tile([P, QT], f32, tag="bq", name=f"bq{l}")
        p["bk"] = small.tile([P, QT], f32, tag="bk", name=f"bk{l}")
        bvr = small.tile([1, DSH], bf16, tag="bvr", name=f"bvr{l}")
        bvrf = small.tile([1, DSH], f32, tag="bvrf", name=f"bvrf{l}")
        p["b1"] = small.tile([P, FMT], f32, tag="b1", name=f"b1{l}")
        nc.sync.dma_start(p["bq"][:], io["bq"][l].rearrange("(m p) -> p m", p=P))
        nc.sync.dma_start(p["bk"][:], io["bk"][l].rearrange("(m p) -> p m", p=P))
        nc.sync.dma_start(bvrf[:], io["bv"][l].rearrange("(o m) -> o m", o=1))
        nc.vector.tensor_copy(bvr[:], bvrf[:])
        nc.sync.dma_start(p["b1"][:], io["b1"][l].rearrange("(m p) -> p m", p=P))
        p["bv"] = bvr
        for nm in ("g1", "be1", "g2", "be2"):
            p[nm] = small.tile([P, DT], f32, tag=nm, name=f"{nm}{l}")
            nc.sync.dma_start(p[nm][:], io[nm][l].rearrange("(k p) -> p k", p=P))
        return p

    # layer-0 loads issued after the embedding instructions so the gather and
    # pes DMAs win queue order; they complete during the transpose phase.
    cur = (load_qk(0), load_wv(0), load_wo(0), load_w1(0), load_w2(0),
           load_params(0))

    # ---------------------------------------- layers
    for l in range(n_layers):
        (wq_sb, wk_sb), wv_sb, wo_sb, w1_sb, w2_sb, prm = cur
        bq_sb, bk_sb, bvr, b1_sb = prm["bq"], prm["bk"], prm["bv"], prm["b1"]
        g1_sb, be1_sb, g2_sb, be2_sb = (prm["g1"], prm["be1"], prm["g2"],
                                        prm["be2"])

        # ---- Q,K projections: [P, QT, SL] = W^T @ x^T (+bias, bf16 out)
        for m in range(QT):
            for nm, src, dst, b_sb in (("q", wq_sb, qT, bq_sb),
                                       ("k", wk_sb, kT, bk_sb)):
                psq = psA.tile([P, SL], f32, tag="mm", name=f"ps{nm}{l}_{m}")
                for k in range(DT):
                    nc.tensor.matmul(psq[:], src[:, k, ts(m, P)], xbf[:, k, :],
                                     start=(k == 0), stop=(k == DT - 1))
                nc.scalar.activation(dst[:, m, :], psq[:], AF.Identity,
                                     bias=b_sb[:, m:m + 1])
        nxt_qk = load_qk(l + 1)

        # ---- V directly in token-major layout: V[tok, ch] = x @ Wv + bv
        for tt in range(KVT):
            psv = psA.tile([P, SL], f32, tag="mm", name=f"psv{l}_{tt}")
            nc.tensor.matmul(psv[:, 0:DSH], onesb[0:1, 0:P], bvr[:],
                             start=True, stop=False)
            for k in range(DT):
                nc.tensor.matmul(psv[:, 0:DSH], xbf[:, k, ts(tt, P)],
                                 wv_sb[:, k, :],
                                 start=False, stop=(k == DT - 1))
            nc.vector.tensor_copy(
                vsb[:, tt, :].rearrange("p (h x) -> p h x", x=HD + 1)
                [:, :, 0:HD],
                psv[:, 0:DSH].rearrange("p (h x) -> p h x", x=HD))
        nxt_wv = load_wv(l + 1)

        # ---- attention, two passes so no PE matmul waits on the DVE
        # pass 1: per head, scores -> exp -> U (+denominator row), copy U to
        # SBUF bf16, reciprocal of the denominator in place. No PE op here
        # depends on the reciprocal, so heads stream through the PE.
        uhs = []
        for h in range(HPC):
            qt_, prow = h // 2, (h % 2) * HD
            vof = h * (HD + 1)
            ets = []
            for kvt in range(KVT):
                pss = psS.tile([P, SL], f32, tag="ss", name=f"pss{l}_{h}_{kvt}")
                nc.tensor.matmul(
                    pss[:],
                    kT[prow:prow + HD, qt_, ts(kvt, P)],
                    qT[prow:prow + HD, qt_, :],
                    start=True, stop=True)
                et = epool.tile([P, SL], bf16, tag="e", name=f"et{l}_{h}_{kvt}")
                nc.scalar.activation(et[:], pss[:], AF.Exp, scale=att_scale)
                ets.append(et)
            psu = psUR.tile([P, SL], f32, tag="ur", name=f"psu{l}_{h}")
            for kvt in range(KVT):
                nc.tensor.matmul(psu[0:HD + 1, :],
                                 vsb[:, kvt, vof:vof + HD + 1],
                                 ets[kvt][:],
                                 start=(kvt == 0), stop=(kvt == KVT - 1))
            uh = upool.tile([P, SL], bf16, tag="u", name=f"uh{l}_{h}")
            nc.scalar.copy(uh[0:HD + 1, :], psu[0:HD + 1, :])
            with nc.allow_low_precision(reason="softmax denom recip in bf16"):
                nc.vector.reciprocal(uh[HD:HD + 1, :], uh[HD:HD + 1, :])
            uhs.append(uh)
        # pass 2: broadcast each head's reciprocal row across 64 partitions
        # via a rank-1 PE matmul (recips all landed during pass 1) and scale.
        for h in range(HPC):
            qt_, prow = h // 2, (h % 2) * HD
            uh = uhs[h]
            psr = psUR.tile([P, SL], f32, tag="ur", name=f"psr{l}_{h}")
            nc.tensor.matmul(psr[0:HD, :], onesb[HD:HD + 1, 0:HD],
                             uh[HD:HD + 1, :], start=True, stop=True)
            nc.vector.tensor_tensor(out=attnT[prow:prow + HD, qt_, :],
                                    in0=uh[0:HD, :],
                                    in1=psr[0:HD, :], op=ALU.mult)

        nxt_prm = load_params(l + 1)

        # ---- Wo partial -> chunked pair AR -> BN1
        ar1 = partial_to_ar(f"o{l}", wo_sb, QT, lambda kt: attnT[:, kt, :],
                            ardt=bf16)
        nxt_wo = load_wo(l + 1)
        batchnorm(f"a{l}", ar1, g1_sb, be1_sb)

        # ---- FFN
        ht = hpool.tile([P, FMT, SL], bf16, tag="ht", name=f"ht{l}")
        for m in range(FMT):
            ps1 = psA.tile([P, SL], f32, tag="mm", name=f"ps1{l}_{m}")
            for k in range(DT):
                nc.tensor.matmul(ps1[:], w1_sb[:, k, ts(m, P)], xbf[:, k, :],
                                 start=(k == 0), stop=(k == DT - 1))
            nc.scalar.activation(ht[:, m, :], ps1[:], AF.Relu,
                                 bias=b1_sb[:, m:m + 1])
        nxt_w1 = load_w1(l + 1)
        ar2 = partial_to_ar(f"f{l}", w2_sb, FMT, lambda kt: ht[:, kt, :])
        nxt_w2 = load_w2(l + 1)
        batchnorm(f"f{l}", ar2, g2_sb, be2_sb)
        cur = (nxt_qk, nxt_wv, nxt_wo, nxt_w1, nxt_w2, nxt_prm)

    # ---------------------------------------- output x^T -> [D, SL] (fp32)
    nc.sync.dma_start(io["out"].rearrange("(k p) t -> p k t", p=P), xbuf[:])
    st.close()


# ================================================================ host side

def _bf(a):
    import ml_dtypes
    return np.ascontiguousarray(np.asarray(a, dtype=np.float32)
                                .astype(ml_dtypes.bfloat16))


def make_in_maps(inputs):
    f = lambda a: np.ascontiguousarray(np.asarray(a), dtype=np.float32)
    seq = np.asarray(inputs["sequence"]).astype(np.int16)       # [B, S]
    emb = f(inputs["emb"])
    pesT = np.ascontiguousarray(np.asarray(inputs["pes"], dtype=np.float32).T)
    Wq, Wk, Wv = (np.asarray(inputs[k]) for k in ("Wq", "Wk", "Wv"))
    Wo, W1, W2 = (np.asarray(inputs[k]) for k in ("Wo", "W1", "W2"))
    bq, bk, bv = f(inputs["bq"]), f(inputs["bk"]), f(inputs["bv"])
    b1 = f(inputs["b1"])
    g1, be1 = f(inputs["g1"]), f(inputs["be1"])
    g2, be2 = f(inputs["g2"]), f(inputs["be2"])

    in_maps = []
    for c in range(NC):
        b, t = c // TP, c % TP
        ds_ = slice(t * DSH, (t + 1) * DSH)
        fs_ = slice(t * FSH, (t + 1) * FSH)
        idx = np.ascontiguousarray(seq[b].reshape(SL // 16, 16).T)  # [16, 32]
        in_maps.append({
            "emb": emb,
            "idx": idx,
            "pesT": pesT,
            "wq": _bf(Wq[:, :, ds_]),
            "wk": _bf(Wk[:, :, ds_]),
            "wv": _bf(Wv[:, :, ds_]),
            "wo": _bf(Wo[:, ds_, :]),
            "w1": _bf(W1[:, :, fs_]),
            "w2": _bf(W2[:, fs_, :]),
            "bq": np.ascontiguousarray(bq[:, ds_]),
            "bk": np.ascontiguousarray(bk[:, ds_]),
            "bv": np.ascontiguousarray(bv[:, ds_]),
            "b1": np.ascontiguousarray(b1[:, fs_]),
            "g1": g1, "be1": be1, "g2": g2, "be2": be2,
        })
    return in_maps


def assemble(results):
    """[B,S,D] fp32 from per-core [D,SL] outs (cores 0,2,4,6)."""
    outs = []
    for b in range(B):
        o = np.asarray(results[TP * b]["out"]).astype(np.float32)  # [D, SL]
        outs.append(np.ascontiguousarray(o.T))                     # [SL, D]
    return np.stack(outs, axis=0)


_CACHE = {}


def _get_module():
    if "nc" not in _CACHE:
        _CACHE["nc"] = build_module()
    return _CACHE["nc"]


def kernel(**inputs):
    from concourse import bass_utils
    nc = _get_module()
    in_maps = make_in_maps(inputs)
    res = bass_utils.run_bass_kernel_spmd(nc, in_maps, list(range(NC)))
    return assemble(res.results)


# revision 18
# speedup vs baseline: 1.2152x; 1.0564x over previous
"""Trainium2 Bass kernel for a 6-layer post-BatchNorm transformer encoder.

Reference model:
  x = emb[seq] + pes                                  # [B,S,D] = [4,512,1024]
  6x: x = BN(x + attn(x)); x = BN(x + ffn(x))
  BN = per-channel batch stats over (B,S), eps=1e-3.

Sharding: dp=4 x tp=2 mesh over 8 NeuronCores. Core c owns sample b=c//2
(512 tokens) and tensor-parallel half t=c%2 (8 heads of QKV/Wo, 2048 of the
4096 FFN hidden units). Per sublayer the pair AllReduces its partial [D,512]
output in two half-token chunks (bf16, 0.5MB each) so the first chunk's
reduce overlaps the second chunk's matmuls, and the residual-add plus
BN-stat computation of chunk 0 overlaps the reduce of chunk 1. BatchNorm
batch statistics are combined with an 8KB 8-core AllReduce (each sample
counted twice -> divide by 2T).

x is kept in fp32 (residual adds, BN stats and apply are exact); a bf16 copy
feeds the PE. Weights are bf16 (host-converted). V is produced directly in
token-major layout by using x-tiles as the stationary operand (no PE
transposes); its bias comes from a ones-row rank-1 matmul. Attention per
head: scores^T = K_h @ Q_h^T, E = exp(scale*scores^T), U^T = V_h^T @ E^T
with softmax denominators accumulated via a ones-column appended to V,
inverted with the 1-instruction approx reciprocal and broadcast across
partitions by a rank-1 PE matmul.

Host side shards inputs per core and reassembles the 4 samples from cores
0,2,4,6 - no final gather collective.
"""

import os

import numpy as np

import concourse.bass as bass
import concourse.mybir as mybir
import concourse.tile as tile
from concourse import bacc
from concourse.bass import ts
from concourse.masks import make_identity

# ---------------------------------------------------------------- dims
V, D, L, H, B, S = 32000, 1024, 6, 16, 4, 512
HD = D // H            # 64
DF = 4 * D             # 4096
EPS = 1e-3
NC = 8                 # cores
P = 128                # partitions
T = B * S              # 2048 tokens total
SL = S                 # tokens per core (one sample)
CH2 = SL // 2          # AR chunk = 256 tokens
DT = D // P            # 8 d-tiles
TP = 2                 # tensor-parallel width
DSH = D // TP          # qkv out shard = 512
QT = DSH // P          # 4 q-tiles
HPC = H // TP          # heads per core = 8
FSH = DF // TP         # ffn hidden shard = 2048
FMT = FSH // P         # ffn1 m-tiles = 16
KVT = SL // P          # kv token tiles = 4

f32 = mybir.dt.float32
bf16 = mybir.dt.bfloat16
i16 = mybir.dt.int16
AF = mybir.ActivationFunctionType
ALU = mybir.AluOpType

PAIRS = [[0, 1], [2, 3], [4, 5], [6, 7]]
ALL8 = [list(range(NC))]

N_LAYERS = int(os.environ.get("TRN_KERNEL_LAYERS", str(L)))


def build_module(n_layers=None):
    if n_layers is None:
        n_layers = N_LAYERS
    nc = bacc.Bacc("TRN2", target_bir_lowering=False, debug=False,
                   num_devices=NC)

    dt_ = nc.dram_tensor
    io = {
        "emb": dt_("emb", [V, D], f32, kind="ExternalInput").ap(),
        "idx": dt_("idx", [16, SL // 16], i16, kind="ExternalInput").ap(),
        "pesT": dt_("pesT", [D, SL], f32, kind="ExternalInput").ap(),
        "wq": dt_("wq", [L, D, DSH], bf16, kind="ExternalInput").ap(),
        "wk": dt_("wk", [L, D, DSH], bf16, kind="ExternalInput").ap(),
        "wv": dt_("wv", [L, D, DSH], bf16, kind="ExternalInput").ap(),
        "wo": dt_("wo", [L, DSH, D], bf16, kind="ExternalInput").ap(),
        "w1": dt_("w1", [L, D, FSH], bf16, kind="ExternalInput").ap(),
        "w2": dt_("w2", [L, FSH, D], bf16, kind="ExternalInput").ap(),
        "bq": dt_("bq", [L, DSH], f32, kind="ExternalInput").ap(),
        "bk": dt_("bk", [L, DSH], f32, kind="ExternalInput").ap(),
        "bv": dt_("bv", [L, DSH], f32, kind="ExternalInput").ap(),
        "b1": dt_("b1", [L, FSH], f32, kind="ExternalInput").ap(),
        "g1": dt_("g1", [L, D], f32, kind="ExternalInput").ap(),
        "be1": dt_("be1", [L, D], f32, kind="ExternalInput").ap(),
        "g2": dt_("g2", [L, D], f32, kind="ExternalInput").ap(),
        "be2": dt_("be2", [L, D], f32, kind="ExternalInput").ap(),
        "out": dt_("out", [D, SL], f32, kind="ExternalOutput").ap(),
    }

    with tile.TileContext(nc) as tc:
        _build(tc, n_layers, io)
    nc.compile()
    return nc


def _build(tc, n_layers, io):
    from contextlib import ExitStack
    nc = tc.nc
    att_scale = 1.0 / np.sqrt(HD)

    # ------------------------------------------------ pools
    st = ExitStack()
    persist = st.enter_context(tc.tile_pool(name="persist", bufs=1))
    wqkv = st.enter_context(tc.tile_pool(name="wqkv", bufs=1))
    wff = st.enter_context(tc.tile_pool(name="wff", bufs=1))
    small = st.enter_context(tc.tile_pool(name="small", bufs=2))
    ybuf = st.enter_context(tc.tile_pool(name="ybuf", bufs=2))   # AR readback
    gbuf = st.enter_context(tc.tile_pool(name="gbuf", bufs=1))   # emb gather
    epool = st.enter_context(tc.tile_pool(name="epool", bufs=5))
    upool = st.enter_context(tc.tile_pool(name="upool", bufs=9))
    hpool = st.enter_context(tc.tile_pool(name="hpool", bufs=1))  # ffn hidden
    psA = st.enter_context(tc.tile_pool(name="psA", bufs=3, space="PSUM"))
    psS = st.enter_context(tc.tile_pool(name="psS", bufs=3, space="PSUM"))
    psUR = st.enter_context(tc.tile_pool(name="psUR", bufs=2, space="PSUM"))
    drin = st.enter_context(tc.tile_pool(name="drin", bufs=4, space="DRAM"))
    drout = st.enter_context(tc.tile_pool(name="drout", bufs=4, space="DRAM"))
    drst = st.enter_context(tc.tile_pool(name="drst", bufs=2, space="DRAM"))

    # ------------------------------------------------ persistent tiles
    xbuf = persist.tile([P, DT, SL], f32, name="xbuf")     # x^T (fp32)
    xbf = persist.tile([P, DT, SL], bf16, name="xbf")      # x^T (bf16 copy)
    qT = persist.tile([P, QT, SL], bf16, name="qT")
    kT = persist.tile([P, QT, SL], bf16, name="kT")
    vsb = persist.tile([P, KVT, HPC * (HD + 1)], bf16, name="vsb")
    attnT = persist.tile([P, QT, SL], bf16, name="attnT")
    ident = persist.tile([P, P], f32, name="ident")
    ones1 = persist.tile([P, P], f32, name="ones1")
    onesb = persist.tile([P, P], bf16, name="onesb")
    idxs = persist.tile([P, SL // 16], i16, name="idxs")

    make_identity(nc, ident[:])
    nc.vector.memset(ones1[:], 1.0)
    nc.vector.memset(onesb[:], 1.0)
    # ones columns of vsb (col HD of each head block), set once
    for h in range(HPC):
        nc.scalar.activation(vsb[:, :, h * (HD + 1) + HD:h * (HD + 1) + HD + 1],
                             ident[:, 0:KVT].unsqueeze(-1),
                             AF.Identity, bias=1.0, scale=0.0)
    for r_ in range(P // 16):
        nc.sync.dma_start(idxs[16 * r_:16 * (r_ + 1), :], io["idx"])

    # ---------------------------------------- embedding: x^T = pes^T + (emb[seq])^T
    nc.sync.dma_start(xbuf[:], io["pesT"].rearrange("(k p) s -> p k s", p=P))
    for half in range(KVT // 2):  # gather 256 tokens at a time
        gtile = gbuf.tile([P, 2, D], f32, tag="gt", name=f"gt{half}")
        nc.gpsimd.dma_gather(
            out_ap=gtile[:],
            in_ap=io["emb"],
            idxs_ap=idxs[:, half * 16:(half + 1) * 16],
            num_idxs=2 * P,
            num_idxs_reg=2 * P,
            elem_size=D,
            queue_num=0,
        )
        for j in range(2):
            t = half * 2 + j            # token tile index (= position tile)
            for k in range(DT):
                ptile = psA.tile([P, SL], f32, tag="mm", name=f"tp{t}_{k}")
                nc.tensor.transpose(ptile[:, 0:P], gtile[:, j, ts(k, P)],
                                    ident[:])
                nc.vector.tensor_tensor(
                    out=xbuf[:, k, ts(t, P)],
                    in0=ptile[:, 0:P],
                    in1=xbuf[:, k, ts(t, P)],
                    op=ALU.add,
                )
    nc.vector.tensor_copy(xbf[:], xbuf[:])

    # ------------------- partial-out -> pair AR, chunked over channels
    def partial_to_ar(lbl, w_sb, nk, rhs_ch, ardt=bf16, nchunks=2):
        """chunk c covers out-channel tiles c*MH..c*MH+MH-1 (full tokens);
        earlier chunks' AllReduces overlap later chunks' matmuls."""
        MH = DT // nchunks
        ar_outs = []
        for c in range(nchunks):
            arin = drin.tile([MH * P, SL], ardt, tag=f"ari{MH}",
                             name=f"ari{lbl}_{c}")
            arout = drout.tile([MH * P, SL], ardt, tag=f"aro{MH}",
                               name=f"aro{lbl}_{c}")
            for mi in range(MH):
                m = c * MH + mi
                ps2 = psA.tile([P, SL], f32, tag="mm", name=f"o{lbl}_{c}_{mi}")
                for kt in range(nk):
                    nc.tensor.matmul(ps2[:], w_sb[:, kt, ts(m, P)],
                                     rhs_ch(kt),
                                     start=(kt == 0), stop=(kt == nk - 1))
                osb = epool.tile([P, SL], ardt, tag="e",
                                 name=f"ob{lbl}_{c}_{mi}")
                nc.vector.tensor_copy(osb[:], ps2[:])
                nc.sync.dma_start(arin[ts(mi, P), :], osb[:])
            nc.gpsimd.collective_compute(
                "AllReduce", ALU.add, replica_groups=PAIRS,
                ins=[arin.opt()], outs=[arout.opt()])
            ar_outs.append(arout)
        return ar_outs

    # ---------------------------------------- batchnorm (channel chunks)
    def batchnorm(lbl, ar_outs, g_sb, be_sb):
        MH = DT // len(ar_outs)
        stats = small.tile([P, 2 * DT], f32, tag="st", name=f"st{lbl}")
        for c, arout in enumerate(ar_outs):
            ydt = arout.tensor.dtype
            yt = ybuf.tile([P, MH, SL], ydt, tag=f"yt{MH}{ydt != bf16}",
                           name=f"yt{lbl}{c}")
            nc.sync.dma_start(yt[:], arout.rearrange("(k p) t -> p k t", p=P))
            xs = xbuf[:, c * MH:(c + 1) * MH, :]
            nc.vector.tensor_tensor(out=xs, in0=xs, in1=yt[:], op=ALU.add)
            nc.vector.reduce_sum(
                out=stats[:, c * MH:(c + 1) * MH].unsqueeze(-1), in_=xs,
                axis=mybir.AxisListType.X)
            for ki in range(MH):
                k = c * MH + ki
                scr = epool.tile([P, SL], bf16, tag="e", name=f"sq{lbl}{c}_{ki}")
                nc.scalar.activation(scr[:], xbuf[:, k, :], AF.Square,
                                     accum_out=stats[:, DT + k:DT + k + 1])
        sin = drst.tile([P, 2 * DT], f32, tag="si", name=f"si{lbl}")
        sout = drst.tile([P, 2 * DT], f32, tag="so", addr_space="Shared",
                         name=f"so{lbl}")
        nc.sync.dma_start(sin, stats[:])
        nc.gpsimd.collective_compute(
            "AllReduce", ALU.add, replica_groups=ALL8,
            ins=[sin.opt()], outs=[sout.opt()])
        gstats = small.tile([P, 2 * DT], f32, tag="gs", name=f"gs{lbl}")
        nc.sync.dma_start(gstats[:], sout)
        # finalize: mean/var over 2T (each sample contributed twice)
        mean = small.tile([P, DT], f32, tag="mean", name=f"mean{lbl}")
        nc.vector.tensor_scalar_mul(mean[:], gstats[:, 0:DT], 1.0 / (2 * T))
        msq = small.tile([P, DT], f32, tag="msq", name=f"msq{lbl}")
        nc.vector.tensor_tensor(out=msq[:], in0=mean[:], in1=mean[:], op=ALU.mult)
        veps = small.tile([P, DT], f32, tag="veps", name=f"veps{lbl}")
        nc.vector.scalar_tensor_tensor(out=veps[:], in0=gstats[:, DT:2 * DT],
                                       scalar=1.0 / (2 * T),
                                       in1=msq[:], op0=ALU.mult, op1=ALU.subtract)
        nc.vector.tensor_scalar_add(veps[:], veps[:], EPS)
        rec = small.tile([P, DT], f32, tag="rec", name=f"rec{lbl}")
        nc.vector.reciprocal(rec[:], veps[:])
        rstd = small.tile([P, DT], f32, tag="rstd", name=f"rstd{lbl}")
        nc.scalar.sqrt(rstd[:], rec[:])
        sc = small.tile([P, DT], f32, tag="sc", name=f"sc{lbl}")
        nc.vector.tensor_tensor(out=sc[:], in0=g_sb[:], in1=rstd[:], op=ALU.mult)
        sh = small.tile([P, DT], f32, tag="sh", name=f"sh{lbl}")
        nc.vector.tensor_tensor(out=sh[:], in0=mean[:], in1=sc[:], op=ALU.mult)
        nc.vector.tensor_tensor(out=sh[:], in0=be_sb[:], in1=sh[:], op=ALU.subtract)
        for k in range(DT):
            nc.scalar.activation(xbuf[:, k, :], xbuf[:, k, :], AF.Identity,
                                 bias=sh[:, k:k + 1], scale=sc[:, k:k + 1])
            nc.vector.tensor_copy(xbf[:, k, :], xbuf[:, k, :])

    # ---------------------------------------- weight/param loaders
    def load_qk(l):
        if l >= n_layers:
            return None
        wq_sb = wqkv.tile([P, DT, DSH], bf16, tag="wq", name=f"wq{l}")
        wk_sb = wqkv.tile([P, DT, DSH], bf16, tag="wk", name=f"wk{l}")
        nc.sync.dma_start(wq_sb[:], io["wq"][l].rearrange("(k p) m -> p k m", p=P))
        nc.sync.dma_start(wk_sb[:], io["wk"][l].rearrange("(k p) m -> p k m", p=P))
        return wq_sb, wk_sb

    def load_wv(l):
        if l >= n_layers:
            return None
        wv_sb = wqkv.tile([P, DT, DSH], bf16, tag="wv", name=f"wv{l}")
        nc.sync.dma_start(wv_sb[:], io["wv"][l].rearrange("(k p) m -> p k m", p=P))
        return wv_sb

    def load_wo(l):
        if l >= n_layers:
            return None
        wo_sb = wqkv.tile([P, QT, D], bf16, tag="wo", name=f"wo{l}")
        nc.sync.dma_start(wo_sb[:], io["wo"][l].rearrange("(k p) m -> p k m", p=P))
        return wo_sb

    def load_w1(l):
        if l >= n_layers:
            return None
        w1_sb = wff.tile([P, DT, FSH], bf16, tag="w1", name=f"w1{l}")
        nc.sync.dma_start(w1_sb[:], io["w1"][l].rearrange("(k p) m -> p k m", p=P))
        return w1_sb

    def load_w2(l):
        if l >= n_layers:
            return None
        w2_sb = wff.tile([P, FMT, D], bf16, tag="w2", name=f"w2{l}")
        nc.sync.dma_start(w2_sb[:], io["w2"][l].rearrange("(k p) m -> p k m", p=P))
        return w2_sb

    def load_params(l):
        if l >= n_layers:
            return None
        p = {}
        p["bq"] = small.tile([P, QT], f32, tag="bq", name=f"bq{l}")
        p["bk"] = small.tile([P, QT], f32, tag="bk", name=f"bk{l}")
        bvr = small.tile([1, DSH], bf16, tag="bvr", name=f"bvr{l}")
        bvrf = small.tile([1, DSH], f32, tag="bvrf", name=f"bvrf{l}")
        p["b1"] = small.tile([P, FMT], f32, tag="b1", name=f"b1{l}")
        nc.sync.dma_start(p["bq"][:], io["bq"][l].rearrange("(m p) -> p m", p=P))
        nc.sync.dma_start(p["bk"][:], io["bk"][l].rearrange("(m p) -> p m", p=P))
        nc.sync.dma_start(bvrf[:], io["bv"][l].rearrange("(o m) -> o m", o=1))
        nc.vector.tensor_copy(bvr[:], bvrf[:])
        nc.sync.dma_start(p["b1"][:], io["b1"][l].rearrange("(m p) -> p m", p=P))
        p["bv"] = bvr
        for nm in ("g1", "be1", "g2", "be2"):
            p[nm] = small.tile([P, DT], f32, tag=nm, name=f"{nm}{l}")
            nc.sync.dma_start(p[nm][:], io[nm][l].rearrange("(k p) -> p k", p=P))
        return p

    # layer-0 loads issued after the embedding instructions so the gather and
    # pes DMAs win queue order; they complete during the transpose phase.
    cur = (load_qk(0), load_wv(0), load_wo(0), load_w1(0), load_w2(0),
           load_params(0))

    # ---------------------------------------- layers
    for l in range(n_layers):
        (wq_sb, wk_sb), wv_sb, wo_sb, w1_sb, w2_sb, prm = cur
        bq_sb, bk_sb, bvr, b1_sb = prm["bq"], prm["bk"], prm["bv"], prm["b1"]
        g1_sb, be1_sb, g2_sb, be2_sb = (prm["g1"], prm["be1"], prm["g2"],
                                        prm["be2"])

        # ---- Q,K projections: [P, QT, SL] = W^T @ x^T (+bias, bf16 out)
        for m in range(QT):
            for nm, src, dst, b_sb in (("q", wq_sb, qT, bq_sb),
                                       ("k", wk_sb, kT, bk_sb)):
                psq = psA.tile([P, SL], f32, tag="mm", name=f"ps{nm}{l}_{m}")
                for k in range(DT):
                    nc.tensor.matmul(psq[:], src[:, k, ts(m, P)], xbf[:, k, :],
                                     start=(k == 0), stop=(k == DT - 1))
                nc.scalar.activation(dst[:, m, :], psq[:], AF.Identity,
                                     bias=b_sb[:, m:m + 1])
        nxt_qk = load_qk(l + 1)

        # ---- V directly in token-major layout: V[tok, ch] = x @ Wv + bv
        for tt in range(KVT):
            psv = psA.tile([P, SL], f32, tag="mm", name=f"psv{l}_{tt}")
            nc.tensor.matmul(psv[:, 0:DSH], onesb[0:1, 0:P], bvr[:],
                             start=True, stop=False)
            for k in range(DT):
                nc.tensor.matmul(psv[:, 0:DSH], xbf[:, k, ts(tt, P)],
                                 wv_sb[:, k, :],
                                 start=False, stop=(k == DT - 1))
            nc.vector.tensor_copy(
                vsb[:, tt, :].rearrange("p (h x) -> p h x", x=HD + 1)
                [:, :, 0:HD],
                psv[:, 0:DSH].rearrange("p (h x) -> p h x", x=HD))
        nxt_wv = load_wv(l + 1)

        # ---- attention, two passes so no PE matmul waits on the DVE
        # pass 1: per head, scores -> exp -> U (+denominator row), copy U to
        # SBUF bf16, reciprocal of the denominator in place. No PE op here
        # depends on the reciprocal, so heads stream through the PE.
        uhs = []
        for h in range(HPC):
            qt_, prow = h // 2, (h % 2) * HD
            vof = h * (HD + 1)
            ets = []
            for kvt in range(KVT):
                pss = psS.tile([P, SL], f32, tag="ss", name=f"pss{l}_{h}_{kvt}")
                nc.tensor.matmul(
                    pss[:],
                    kT[prow:prow + HD, qt_, ts(kvt, P)],
                    qT[prow:prow + HD, qt_, :],
                    start=True, stop=True)
                et = epool.tile([P, SL], bf16, tag="e", name=f"et{l}_{h}_{kvt}")
                nc.scalar.activation(et[:], pss[:], AF.Exp, scale=att_scale)
                ets.append(et)
            psu = psUR.tile([P, SL], f32, tag="ur", name=f"psu{l}_{h}")
            for kvt in range(KVT):
                nc.tensor.matmul(psu[0:HD + 1, :],
                                 vsb[:, kvt, vof:vof + HD + 1],
                                 ets[kvt][:],
                                 start=(kvt == 0), stop=(kvt == KVT - 1))
            uh = upool.tile([P, SL], bf16, tag="u", name=f"uh{l}_{h}")
            nc.scalar.copy(uh[0:HD + 1, :], psu[0:HD + 1, :])
            with nc.allow_low_precision(reason="softmax denom recip in bf16"):
                nc.vector.reciprocal(uh[HD:HD + 1, :], uh[HD:HD + 1, :])
            uhs.append(uh)
        # pass 2: broadcast each head's reciprocal row across 64 partitions
        # via a rank-1 PE matmul (recips all landed during pass 1) and scale.
        for h in range(HPC):
            qt_, prow = h // 2, (h % 2) * HD
            uh = uhs[h]
            psr = psUR.tile([P, SL], f32, tag="ur", name=f"psr{l}_{h}")
            nc.tensor.matmul(psr[0:HD, :], onesb[HD:HD + 1, 0:HD],
                             uh[HD:HD + 1, :], start=True, stop=True)
            nc.vector.tensor_tensor(out=attnT[prow:prow + HD, qt_, :],
                                    in0=uh[0:HD, :],
                                    in1=psr[0:HD, :], op=ALU.mult)

        nxt_prm = load_params(l + 1)

        # ---- Wo partial -> chunked pair AR -> BN1
        ar1 = partial_to_ar(f"o{l}", wo_sb, QT, lambda kt: attnT[:, kt, :],
                            ardt=bf16, nchunks=1)
        nxt_wo = load_wo(l + 1)
        batchnorm(f"a{l}", ar1, g1_sb, be1_sb)

        # ---- FFN
        ht = hpool.tile([P, FMT, SL], bf16, tag="ht", name=f"ht{l}")
        for m in range(FMT):
            ps1 = psA.tile([P, SL], f32, tag="mm", name=f"ps1{l}_{m}")
            for k in range(DT):
                nc.tensor.matmul(ps1[:], w1_sb[:, k, ts(m, P)], xbf[:, k, :],
                                 start=(k == 0), stop=(k == DT - 1))
            nc.scalar.activation(ht[:, m, :], ps1[:], AF.Relu,
                                 bias=b1_sb[:, m:m + 1])
        nxt_w1 = load_w1(l + 1)
        ar2 = partial_to_ar(f"f{l}", w2_sb, FMT, lambda kt: ht[:, kt, :])
        nxt_w2 = load_w2(l + 1)
        batchnorm(f"f{l}", ar2, g2_sb, be2_sb)
        cur = (nxt_qk, nxt_wv, nxt_wo, nxt_w1, nxt_w2, nxt_prm)

    # ---------------------------------------- output x^T -> [D, SL] (fp32)
    nc.sync.dma_start(io["out"].rearrange("(k p) t -> p k t", p=P), xbuf[:])
    st.close()


# ================================================================ host side

def _bf(a):
    import ml_dtypes
    return np.ascontiguousarray(np.asarray(a, dtype=np.float32)
                                .astype(ml_dtypes.bfloat16))


def make_in_maps(inputs):
    f = lambda a: np.ascontiguousarray(np.asarray(a), dtype=np.float32)
    seq = np.asarray(inputs["sequence"]).astype(np.int16)       # [B, S]
    emb = f(inputs["emb"])
    pesT = np.ascontiguousarray(np.asarray(inputs["pes"], dtype=np.float32).T)
    Wq, Wk, Wv = (np.asarray(inputs[k]) for k in ("Wq", "Wk", "Wv"))
    Wo, W1, W2 = (np.asarray(inputs[k]) for k in ("Wo", "W1", "W2"))
    bq, bk, bv = f(inputs["bq"]), f(inputs["bk"]), f(inputs["bv"])
    b1 = f(inputs["b1"])
    g1, be1 = f(inputs["g1"]), f(inputs["be1"])
    g2, be2 = f(inputs["g2"]), f(inputs["be2"])

    in_maps = []
    for c in range(NC):
        b, t = c // TP, c % TP
        ds_ = slice(t * DSH, (t + 1) * DSH)
        fs_ = slice(t * FSH, (t + 1) * FSH)
        idx = np.ascontiguousarray(seq[b].reshape(SL // 16, 16).T)  # [16, 32]
        in_maps.append({
            "emb": emb,
            "idx": idx,
            "pesT": pesT,
            "wq": _bf(Wq[:, :, ds_]),
            "wk": _bf(Wk[:, :, ds_]),
            "wv": _bf(Wv[:, :, ds_]),
            "wo": _bf(Wo[:, ds_, :]),
            "w1": _bf(W1[:, :, fs_]),
            "w2": _bf(W2[:, fs_, :]),
            "bq": np.ascontiguousarray(bq[:, ds_]),
            "bk": np.ascontiguousarray(bk[:, ds_]),
            "bv": np.ascontiguousarray(bv[:, ds_]),
            "b1": np.ascontiguousarray(b1[:, fs_]),
            "g1": g1, "be1": be1, "g2": g2, "be2": be2,
        })
    return in_maps


def assemble(results):
    """[B,S,D] fp32 from per-core [D,SL] outs (cores 0,2,4,6)."""
    outs = []
    for b in range(B):
        o = np.asarray(results[TP * b]["out"]).astype(np.float32)  # [D, SL]
        outs.append(np.ascontiguousarray(o.T))                     # [SL, D]
    return np.stack(outs, axis=0)


_CACHE = {}


def _get_module():
    if "nc" not in _CACHE:
        _CACHE["nc"] = build_module()
    return _CACHE["nc"]


def kernel(**inputs):
    from concourse import bass_utils
    nc = _get_module()
    in_maps = make_in_maps(inputs)
    res = bass_utils.run_bass_kernel_spmd(nc, in_maps, list(range(NC)))
    return assemble(res.results)
